# revision 1
# baseline (speedup 1.0000x reference)
"""Trainium2 Bass kernel for nn_MultiHeadAttn_80126909874682.

Full MHA layer: QKV projection -> 16-head attention (seq 2048) -> output
projection -> residual -> LayerNorm, over h [2048, 4, 1024] fp32.

Sharding (8 NeuronCores, zero collectives):
  core c -> batch b = c // 2, token-half r = c % 2.
  Each core computes K/V for all 2048 tokens of its batch (all 16 heads)
  and Q / attention / output projection / LayerNorm for its 1024 local
  tokens only.  The per-core `hb` input is permuted so the core's local
  tokens come first; attention is invariant to the j-permutation of K/V,
  so the program stays uniform SPMD while the data differs per core.

Structure (v2, pipelined): head-pair-major loop — pair p's K/Q
projections are emitted just before its attention, so the TensorEngine
work of pair p+1's projections hides under pair p's softmax (ACT) work.
V is produced in two batches (heads 0-7, 8-15) reusing one half-sized
weight buffer.  Scores use 2-bank PSUM tiles so each Exp activation
covers 1024 elements/partition (halves ACT instruction overhead).

Compute dtypes: matmul operands bf16 (weights pre-converted on host),
PSUM accumulation / softmax statistics / LayerNorm in fp32.
Softmax uses exp without max-subtraction (scores are O(1) by
construction) and a ones-column appended to V so the PV matmul also
produces the softmax denominators.
"""

import os
import sys

os.environ.setdefault("JAX_PLATFORMS", "axon")
sys.path.insert(0, "/opt/trn_rl_repo")

import numpy as np
import ml_dtypes

import concourse.bass as bass
import concourse.tile as tile
from concourse import bacc, mybir
from concourse.bass import ts
from concourse.bass_utils import run_bass_kernel_spmd
from concourse.masks import make_identity

N_HEAD = 16
D_MODEL = 1024
D_HEAD = 64
SEQ = 2048
BATCH = 4
EPS = 1e-5
N_CORES = 8

LOCAL = SEQ // 2            # tokens owned per core (1024)
N_PAIR = N_HEAD // 2        # head pairs (8)
CC = D_MODEL // 128         # contraction chunks (8)
JT = SEQ // 128             # j tiles (16)
JG = JT // 2                # j tile pairs (8)
IT_ALL = SEQ // 128         # token tiles for transpose (16)
IB_ALL = SEQ // 512         # 512-token blocks, all tokens (4)
IB_LOC = LOCAL // 512       # 512-token blocks, local tokens (2)
ISUB = LOCAL // 128         # 128-token sub tiles, local (8)

F32 = mybir.dt.float32
BF16 = mybir.dt.bfloat16
AF = mybir.ActivationFunctionType


def build_program():
    nc = bacc.Bacc()

    hb = nc.declare_dram_parameter("hb", [SEQ, D_MODEL], F32, isOutput=False)
    hbt_d = nc.declare_dram_parameter("hbt", [D_MODEL, SEQ], BF16, isOutput=False)
    wq = nc.declare_dram_parameter("wq", [D_MODEL, D_MODEL], BF16, isOutput=False)
    wk = nc.declare_dram_parameter("wk", [D_MODEL, D_MODEL], BF16, isOutput=False)
    wv = nc.declare_dram_parameter("wv", [D_MODEL, D_MODEL], BF16, isOutput=False)
    wo = nc.declare_dram_parameter("wo", [D_MODEL, D_MODEL], BF16, isOutput=False)
    gamma = nc.declare_dram_parameter("gamma", [D_MODEL], F32, isOutput=False)
    beta = nc.declare_dram_parameter("beta", [D_MODEL], F32, isOutput=False)
    out = nc.declare_dram_parameter("out", [LOCAL, D_MODEL], F32, isOutput=True)

    with tile.TileContext(nc) as tc:
        with (
            tc.tile_pool(name="consts", bufs=1) as consts,
            tc.tile_pool(name="wo_w", bufs=1) as wo_pool,
            tc.tile_pool(name="hbt", bufs=1) as hbt_pool,
            tc.tile_pool(name="w_qk", bufs=1) as wqk_pool,
            tc.tile_pool(name="w_v", bufs=1) as wv_pool,
            tc.tile_pool(name="vsb", bufs=1) as v_pool,
            tc.tile_pool(name="ktq", bufs=2) as ktq_pool,
            tc.tile_pool(name="attnT", bufs=1) as attn_pool,
            tc.tile_pool(name="exp", bufs=6) as exp_pool,
            tc.tile_pool(name="small", bufs=2) as rec_pool,
            tc.tile_pool(name="xstage", bufs=3) as x_pool,
            tc.tile_pool(name="hbres", bufs=3) as hbr_pool,
            tc.tile_pool(name="dram", bufs=1, space="DRAM") as dram_pool,
            tc.tile_pool(name="psum", bufs=2, space="PSUM") as psum,
        ):
            _emit(nc, tc, hb, hbt_d, wq, wk, wv, wo, gamma, beta, out,
                  consts, wo_pool, hbt_pool, wqk_pool, wv_pool, v_pool,
                  ktq_pool, attn_pool, exp_pool, rec_pool, x_pool,
                  hbr_pool, dram_pool, psum)

    nc.finalize()
    return nc


def _emit(nc, tc, hb, hbt_d, wq, wk, wv, wo, gamma, beta, out,
          consts, wo_pool, hbt_pool, wqk_pool, wv_pool, v_pool,
          ktq_pool, attn_pool, exp_pool, rec_pool, x_pool,
          hbr_pool, dram_pool, psum):
    # ---- constants ----
    gamma_b = consts.tile([128, D_MODEL], F32)
    beta_b = consts.tile([128, D_MODEL], F32)
    eps_t = consts.tile([128, 1], F32)
    nc.vector.memset(eps_t[:], EPS)

    wo_sb = [wo_pool.tile([128, D_MODEL], BF16, tag=f"wo{c}", name=f"wo{c}")
             for c in range(CC)]

    wq_sb = [wqk_pool.tile([128, D_MODEL], BF16, tag=f"wq{c}", name=f"wq{c}")
             for c in range(CC)]
    wk_sb = [wqk_pool.tile([128, D_MODEL], BF16, tag=f"wk{c}", name=f"wk{c}")
             for c in range(CC)]
    v_sb = [v_pool.tile([128, JT * 65], BF16, tag=f"v{n}", name=f"v{n}")
            for n in range(N_HEAD)]
    for n in range(N_HEAD):
        nc.vector.memset(v_sb[n][:], 1.0)

    # ---- hb^T: host-pre-transposed bf16, straight DMA ----
    hbt = [hbt_pool.tile([128, SEQ], BF16, tag=f"hbt{c}", name=f"hbt{c}")
           for c in range(CC)]
    for cb in range(4):
        for c in range(CC):
            eng = nc.sync if (c + cb) % 2 == 0 else nc.scalar
            eng.dma_start(hbt[c][:, ts(cb, 512)], hbt_d[ts(c, 128), ts(cb, 512)])

    def v_batch(half):
        """Produce V (+ones) for heads 8*half .. 8*half+7."""
        wv_sb = [wv_pool.tile([128, 512], BF16, tag=f"wv{c}", name=f"wv{c}")
                 for c in range(CC)]
        for c in range(CC):
            nc.gpsimd.dma_start(wv_sb[c][:], wv[ts(c, 128), ts(half, 512)])
        for j in range(JT):
            ps = psum.tile([128, 512], F32, tag="ev", name="vps")
            for c in range(CC):
                nc.tensor.matmul(
                    ps[:], hbt[c][:, ts(j, 128)], wv_sb[c][:],
                    start=(c == 0), stop=(c == CC - 1),
                )
            for k in range(8):
                n = 8 * half + k
                nc.vector.tensor_copy(
                    v_sb[n][:, j * 65: j * 65 + 64], ps[:, ts(k, 64)]
                )

    at = {}  # (p, itile) -> attnT tile [128 d, 512 i]

    def wo_block(itile):
        for s4 in range(4):
            isub = 4 * itile + s4
            hbres = hbr_pool.tile([128, D_MODEL], F32, tag="hbres",
                                  name="hbres")
            nc.sync.dma_start(hbres[:], hb[ts(isub, 128), :])
            x = x_pool.tile([128, D_MODEL], F32, tag="x", name="x")
            for dm in range(2):
                ops = psum.tile([128, 512], F32, tag="ev", name="ops")
                for p in range(N_PAIR):
                    nc.tensor.matmul(
                        ops[:], at[(p, itile)][:, ts(s4, 128)],
                        wo_sb[p][:, ts(dm, 512)],
                        start=(p == 0), stop=(p == N_PAIR - 1),
                    )
                nc.vector.tensor_add(
                    x[:, ts(dm, 512)], ops[:], hbres[:, ts(dm, 512)]
                )
            stats = rec_pool.tile([128, 2, 6], F32, tag="bnst", name="st")
            mv = rec_pool.tile([128, 2], F32, tag="bnmv", name="mv")
            for g in range(2):
                nc.vector.bn_stats(stats[:, g, :], x[:, ts(g, 512)])
            nc.vector.bn_aggr(mv[:], stats[:])
            rstd = rec_pool.tile([128, 1], F32, tag="rstd", name="rstd")
            nc.scalar.activation(rstd[:], mv[:, 1:2], AF.Ln, bias=eps_t[:])
            nc.scalar.activation(rstd[:], rstd[:], AF.Exp, scale=-0.5)
            nc.vector.tensor_scalar(
                x[:], x[:], mv[:, 0:1], rstd[:],
                op0=mybir.AluOpType.subtract, op1=mybir.AluOpType.mult,
            )
            nc.vector.tensor_mul(x[:], x[:], gamma_b[:])
            nc.vector.tensor_add(x[:], x[:], beta_b[:])
            nc.sync.dma_start(out[ts(isub, 128), :], x[:])

    for p in range(N_PAIR):
        if p == 0:
            v_batch(0)
            for c in range(CC):
                nc.gpsimd.dma_start(wq_sb[c][:], wq[ts(c, 128), :])
                nc.gpsimd.dma_start(wk_sb[c][:], wk[ts(c, 128), :])
        elif p == 1:
            for c in range(CC):
                nc.scalar.dma_start(wo_sb[c][:], wo[ts(c, 128), :])
            g_ap, b_ap = gamma.ap(), beta.ap()
            nc.gpsimd.dma_start(
                out=gamma_b[:],
                in_=bass.AP(tensor=g_ap.tensor, offset=g_ap.offset,
                            ap=[[0, 128], [1, D_MODEL]]),
            )
            nc.gpsimd.dma_start(
                out=beta_b[:],
                in_=bass.AP(tensor=b_ap.tensor, offset=b_ap.offset,
                            ap=[[0, 128], [1, D_MODEL]]),
            )
        elif p == 4:
            v_batch(1)

        # K^T / Q^T for this pair
        kt_p = ktq_pool.tile([128, SEQ], BF16, tag="kt", name="kt_p")
        for ib in range(IB_ALL):
            ps = psum.tile([128, 512], F32, tag="ev", name="kps")
            for c in range(CC):
                nc.tensor.matmul(
                    ps[:], wk_sb[c][:, ts(p, 128)], hbt[c][:, ts(ib, 512)],
                    start=(c == 0), stop=(c == CC - 1),
                )
            nc.vector.tensor_copy(kt_p[:, ts(ib, 512)], ps[:])
        qt_p = ktq_pool.tile([128, LOCAL], BF16, tag="qt", name="qt_p")
        for ib in range(IB_LOC):
            ps = psum.tile([128, 512], F32, tag="ev", name="qps")
            for c in range(CC):
                nc.tensor.matmul(
                    ps[:], wq_sb[c][:, ts(p, 128)], hbt[c][:, ts(ib, 512)],
                    start=(c == 0), stop=(c == CC - 1),
                )
            nc.vector.tensor_copy(qt_p[:, ts(ib, 512)], ps[:])

        # attention for both local 512-token blocks
        for itile in range(IB_LOC):
            atile = attn_pool.tile([128, 512], BF16,
                                   tag=f"at{p}_{itile}", name="atile")
            at[(p, itile)] = atile
            acc = [psum.tile([128, 512], F32, tag="acc", name="acc")
                   for _ in range(2)]
            for jg in range(JG):
                for h in range(2):
                    n = 2 * p + h
                    s2 = psum.tile([128, 1024], F32, tag="s2", name="s2")
                    for u in range(2):
                        jc = 2 * jg + u
                        nc.tensor.matmul(
                            s2[:, ts(u, 512)],
                            kt_p[ts(h, 64), ts(jc, 128)],
                            qt_p[ts(h, 64), ts(itile, 512)],
                            start=True, stop=True,
                        )
                    e = exp_pool.tile([128, 1024], BF16, tag="e", name="e")
                    nc.scalar.activation(e[:], s2[:], AF.Exp)
                    for u in range(2):
                        jc = 2 * jg + u
                        nc.tensor.matmul(
                            acc[h][0:65, :],
                            v_sb[n][:, jc * 65: jc * 65 + 65],
                            e[:, ts(u, 512)],
                            start=(jg == 0 and u == 0),
                            stop=(jg == JG - 1 and u == 1),
                        )
            for h in range(2):
                rec = rec_pool.tile([1, 512], F32, tag="rec", name="rec")
                nc.vector.reciprocal(rec[:], acc[h][64:65, :])
                rb = rec_pool.tile([64, 512], F32, tag="recb", name="rb")
                nc.gpsimd.partition_broadcast(rb[:], rec[:])
                nc.vector.tensor_mul(
                    atile[ts(h, 64), :], acc[h][0:64, :], rb[:]
                )
            if p == N_PAIR - 1 and itile == 0:
                wo_block(0)

    # itile 1's output projection + LayerNorm (itile 0's was emitted
    # inside the pair loop, right after the last pair finished itile 0)
    wo_block(1)



_program_cache = {}


def _get_program():
    if "nc" not in _program_cache:
        _program_cache["nc"] = build_program()
    return _program_cache["nc"]


def _shard_inputs(h, Wq, Wkv, Wo, gamma, beta):
    """Build the 8 per-core input maps (host-side numpy only)."""
    h = np.asarray(h, np.float32)
    Wq = np.asarray(Wq, np.float32)
    Wkv = np.asarray(Wkv, np.float32)
    Wo = np.asarray(Wo, np.float32)
    gamma = np.asarray(gamma, np.float32)
    beta = np.asarray(beta, np.float32)

    scale = 1.0 / np.sqrt(D_HEAD)
    Wq_s = np.ascontiguousarray((Wq * scale).astype(ml_dtypes.bfloat16))
    Wk = np.ascontiguousarray(Wkv[:, :N_HEAD * D_HEAD].astype(ml_dtypes.bfloat16))
    Wv = np.ascontiguousarray(Wkv[:, N_HEAD * D_HEAD:].astype(ml_dtypes.bfloat16))
    Wo_b = np.ascontiguousarray(Wo.astype(ml_dtypes.bfloat16))

    in_maps = []
    for core in range(N_CORES):
        b, r = divmod(core, 2)
        hb_full = h[:, b, :]  # [2048, 1024]
        if r == 0:
            hb_perm = hb_full
        else:
            hb_perm = np.concatenate([hb_full[LOCAL:], hb_full[:LOCAL]], axis=0)
        hbt_b = np.ascontiguousarray(hb_perm.T.astype(ml_dtypes.bfloat16))
        in_maps.append({
            "hb": np.ascontiguousarray(hb_perm),
            "hbt": hbt_b,
            "wq": Wq_s, "wk": Wk, "wv": Wv, "wo": Wo_b,
            "gamma": gamma, "beta": beta,
        })
    return in_maps


def kernel(h, Wq, Wkv, Wo, gamma, beta, _trace=False):
    nc = _get_program()
    in_maps = _shard_inputs(h, Wq, Wkv, Wo, gamma, beta)
    res = run_bass_kernel_spmd(nc, in_maps, list(range(N_CORES)), trace=_trace)
    if _trace:
        kernel.last_results = res

    out = np.empty((SEQ, BATCH, D_MODEL), np.float32)
    for core in range(N_CORES):
        b, r = divmod(core, 2)
        out[r * LOCAL:(r + 1) * LOCAL, b, :] = res.results[core]["out"]
    return out



# revision 11
# speedup vs baseline: 1.2546x; 1.2546x over previous
"""Trainium2 Bass kernel for nn_MultiHeadAttn_80126909874682.

Full MHA layer: QKV projection -> 16-head attention (seq 2048) -> output
projection -> residual -> LayerNorm, over h [2048, 4, 1024] fp32.

Sharding (8 NeuronCores, zero collectives):
  core c -> batch b = c // 2, token-half r = c % 2.
  Each core computes K/V for all 2048 tokens of its batch (all 16 heads)
  and Q / attention / output projection / LayerNorm for its 1024 local
  tokens only.  The per-core inputs are permuted so the core's local
  tokens come first; attention is invariant to the j-permutation of K/V.

v3 (fp8 DoubleRow): the attention-side matmuls (V projection, QK^T
scores, PV, output projection) run in fp8e4 with the DoubleRow perf
mode (two k-tile slabs contracted per instruction at 0.5 cycles/row).
Q/K projections stay bf16 (fp8 there dominates the output error).  The
scores matmul has only a 64-deep contraction, so its two DoubleRow
slabs alias the same data via stride-0 APs with the 2x folded into the
host-side Wq scale.  Exp runs on the ACT engine with a -ln(16) bias
(keeps e below the fp8e4 max); a ones-column appended to V makes the PV
matmul also emit the softmax denominators.  PSUM: 2x[128,1024] score
tiles + 2x[128,512] PV accumulators + 2x[128,512] projection tiles = 8
banks exactly.  ACT (256 exps) is the critical path; everything else
(copies, normalize, LayerNorm) lives on DVE/Pool/SP.
"""

import os
import sys

os.environ.setdefault("JAX_PLATFORMS", "axon")
sys.path.insert(0, "/opt/trn_rl_repo")

import numpy as np
import ml_dtypes

import concourse.bass as bass
import concourse.tile as tile
from concourse import bacc, mybir
from concourse.bass import ts
from concourse.bass_utils import run_bass_kernel_spmd

N_HEAD = 16
D_MODEL = 1024
D_HEAD = 64
SEQ = 2048
BATCH = 4
EPS = 1e-5
N_CORES = 8

LOCAL = SEQ // 2            # tokens owned per core (1024)
CC = D_MODEL // 128         # dmodel contraction chunks (8)
N_PAIR = N_HEAD // 2        # head pairs (8)
JB = SEQ // 512             # 512-token j blocks (4)
J16 = SEQ // 128            # 128-token j chunks (16)
NU = J16 // 2               # jc pairs per (head, iblock) unit (8)
IBL = LOCAL // 512          # local 512-token i blocks (2)
ISUB = LOCAL // 128         # local 128-token i sub tiles (8)
LN16 = float(np.log(16.0))

F32 = mybir.dt.float32
BF16 = mybir.dt.bfloat16
FP8 = mybir.dt.float8e4
AF = mybir.ActivationFunctionType
DR = mybir.MatmulPerfMode.DoubleRow


def _slab3(ap2, col0, slab_stride, n):
    """[P, n] view at col0 with an extra middle slab dim [slab_stride, 2]."""
    a = ap2[:, col0:col0 + 1]
    return bass.AP(
        tensor=a.tensor, offset=a.offset,
        ap=[list(a.ap[0]), [slab_stride, 2], [1, n]],
    )


def build_program(affine):
    nc = bacc.Bacc()

    hbt16 = nc.declare_dram_parameter("hbt16", [128, JB * CC * 512], BF16,
                                      isOutput=False)
    hbt8 = nc.declare_dram_parameter("hbt8", [128, JB * CC * 512], FP8,
                                     isOutput=False)
    wq16 = nc.declare_dram_parameter("wq16", [128, CC * D_MODEL], BF16,
                                     isOutput=False)
    wk16 = nc.declare_dram_parameter("wk16", [128, CC * D_MODEL], BF16,
                                     isOutput=False)
    wv8 = nc.declare_dram_parameter("wv8", [128, CC * D_MODEL], FP8,
                                    isOutput=False)
    wo8 = nc.declare_dram_parameter("wo8", [128, CC * D_MODEL], FP8,
                                    isOutput=False)
    hb = nc.declare_dram_parameter("hb", [LOCAL, D_MODEL], F32, isOutput=False)
    gamma = nc.declare_dram_parameter("gamma", [D_MODEL], F32, isOutput=False)
    beta = nc.declare_dram_parameter("beta", [D_MODEL], F32, isOutput=False)
    out = nc.declare_dram_parameter("out", [LOCAL, D_MODEL], F32,
                                    isOutput=True)

    with tile.TileContext(nc) as tc:
        with (
            tc.tile_pool(name="consts", bufs=1) as consts,
            tc.tile_pool(name="weights", bufs=1) as wpool,
            tc.tile_pool(name="hbt", bufs=1) as hpool,
            tc.tile_pool(name="ktq", bufs=1) as kpool,
            tc.tile_pool(name="vsb", bufs=1) as vpool,
            tc.tile_pool(name="attn", bufs=1) as apool,
            tc.tile_pool(name="exp", bufs=4) as epool,
            tc.tile_pool(name="small", bufs=3) as spool,
            tc.tile_pool(name="xstage", bufs=2) as xpool,
            tc.tile_pool(name="hbres", bufs=2) as rpool,
            tc.tile_pool(name="psum", bufs=2, space="PSUM") as psum,
        ):
            _emit(nc, hbt16, hbt8, wq16, wk16, wv8, wo8, hb, gamma, beta,
                  out, consts, wpool, hpool, kpool, vpool, apool, epool,
                  spool, xpool, rpool, psum, affine)

    nc.finalize()
    return nc


def _emit(nc, hbt16, hbt8, wq16, wk16, wv8, wo8, hb, gamma, beta, out,
          consts, wpool, hpool, kpool, vpool, apool, epool, spool, xpool,
          rpool, psum, affine):
    # ---- SBUF tiles ----
    eps_t = consts.tile([128, 1], F32)
    nc.vector.memset(eps_t[:], EPS)
    lnb_t = consts.tile([128, 1], F32)
    nc.vector.memset(lnb_t[:], -LN16)
    if affine:
        gamma_b = consts.tile([128, D_MODEL], F32)
        beta_b = consts.tile([128, D_MODEL], F32)
        g_ap, b_ap = gamma.ap(), beta.ap()
        nc.gpsimd.dma_start(
            out=gamma_b[:],
            in_=bass.AP(tensor=g_ap.tensor, offset=g_ap.offset,
                        ap=[[0, 128], [1, D_MODEL]]))
        nc.gpsimd.dma_start(
            out=beta_b[:],
            in_=bass.AP(tensor=b_ap.tensor, offset=b_ap.offset,
                        ap=[[0, 128], [1, D_MODEL]]))

    wk_sb = wpool.tile([128, CC * D_MODEL], BF16, name="wk")
    wq_sb = wpool.tile([128, CC * D_MODEL], BF16, name="wq")
    wv_sb = wpool.tile([128, CC * D_MODEL], FP8, name="wv")
    wo_sb = wpool.tile([128, CC * D_MODEL], FP8, name="wo")
    ht16 = hpool.tile([128, JB * CC * 512], BF16, name="ht16")
    ht8 = hpool.tile([128, JB * CC * 512], FP8, name="ht8")
    # kt: per pair [128p = 2 heads x 64 d] x [pair(8) x j(2048)] fp8
    kt = kpool.tile([128, N_PAIR * SEQ], FP8, name="kt")
    qt = kpool.tile([128, N_PAIR * LOCAL], FP8, name="qt")
    # v8: per head 16 j-chunks x (64 dims + ones col + 15 pad)
    # (DoubleRow slab strides must be multiples of 16 elements)
    v8 = vpool.tile([128, N_HEAD * J16 * 80], FP8, name="v8")
    # at: [128p = pair dims] x [pair(8) x ib(2) x i(512)] fp8
    at = apool.tile([128, 8 * LOCAL], FP8, name="at")

    # ones columns of v8 (col h*1280 + j*80 + 64)
    for n in range(N_HEAD):
        base = n * (J16 * 80) + 64
        a = v8[:, base:base + 1]
        nc.gpsimd.memset(
            bass.AP(tensor=a.tensor, offset=a.offset,
                    ap=[list(a.ap[0]), [80, J16], [1, 1]]), 1.0)

    # ---- DMA loads ----
    # Pool queue: weights (wk first: first K-proj needs all its chunks)
    for c in range(CC):
        nc.gpsimd.dma_start(wk_sb[:, ts(c, 1024)], wk16[:, ts(c, 1024)])
    for c in range(CC):
        nc.gpsimd.dma_start(wq_sb[:, ts(c, 1024)], wq16[:, ts(c, 1024)])
    for c in range(CC // 2):
        nc.gpsimd.dma_start(wv_sb[:, ts(c, 2048)], wv8[:, ts(c, 2048)])
    for c in range(CC // 2):
        nc.gpsimd.dma_start(wo_sb[:, ts(c, 2048)], wo8[:, ts(c, 2048)])
    # SP queue: hbt16 by jb blocks, then hbt8, then residual rows
    for jb in range(JB):
        nc.sync.dma_start(ht16[:, ts(jb, 4096)], hbt16[:, ts(jb, 4096)])
    for jb in range(JB):
        nc.sync.dma_start(ht8[:, ts(jb, 4096)], hbt8[:, ts(jb, 4096)])
    hbres = [rpool.tile([128, D_MODEL], F32, tag=f"hbres{i % 4}",
                        name=f"hbres{i}") for i in range(ISUB)]
    for i in range(ISUB):
        nc.sync.dma_start(hbres[i][:], hb[ts(i, 128), :])

    # ---- emission helpers ----
    def kq_tile(p, jb, is_q):
        """One [128, 512] projection tile of K^T or Q^T (bf16 matmuls).
        Output partitions = (2 heads of pair p) x 64 dims."""
        w, dst, blk = (wq_sb, qt, LOCAL) if is_q else (wk_sb, kt, SEQ)
        ps = psum.tile([128, 512], F32, tag="proj", name="kqps")
        for c in range(CC):
            nc.tensor.matmul(
                ps[:], w[:, c * 1024 + p * 128: c * 1024 + (p + 1) * 128],
                ht16[:, jb * 4096 + c * 512: jb * 4096 + (c + 1) * 512],
                start=(c == 0), stop=(c == CC - 1),
            )
        nc.vector.tensor_copy(dst[:, p * blk + jb * 512:
                                  p * blk + jb * 512 + 512], ps[:])

    def v_tile(j16):
        """V projection for one 128-token j-chunk, all 16 heads (fp8 DR)."""
        jb, t0 = divmod(j16, 4)
        for half in range(2):
            ps = psum.tile([128, 512], F32, tag="proj", name="vps")
            for cp in range(CC // 2):
                lhs = _slab3(ht8[:], jb * 4096 + (2 * cp) * 512 + t0 * 128,
                             512, 128)
                rhs = _slab3(wv_sb[:], (2 * cp) * 1024 + half * 512,
                             1024, 512)
                nc.tensor.matmul(ps[:], lhs, rhs, start=(cp == 0),
                                 stop=(cp == CC // 2 - 1), perf_mode=DR)
            # strided scatter into v8: head n = 8*half+k gets cols
            # n*1280 + j16*80 .. +64
            base = (8 * half) * (J16 * 80) + j16 * 80
            a = v8[:, base:base + 1]
            dst = bass.AP(tensor=a.tensor, offset=a.offset,
                          ap=[list(a.ap[0]), [J16 * 80, 8], [1, 64]])
            nc.vector.tensor_copy(dst, ps[:])

    acc_of = {}

    def unit(h, ib, u):
        """Scores + exp + PV for head h, i-block ib, jc-pair u."""
        p, hh = divmod(h, 2)
        s4 = psum.tile([128, 1024], F32, tag="s4", name="s4")
        for uu in range(2):
            jc = 2 * u + uu
            lhs = _slab3(kt[ts(hh, 64), :], p * SEQ + jc * 128, 0, 128)
            rhs = _slab3(qt[ts(hh, 64), :], p * LOCAL + ib * 512, 0, 512)
            nc.tensor.matmul(s4[:, ts(uu, 512)], lhs, rhs,
                             start=True, stop=True, perf_mode=DR)
        e = epool.tile([128, 1024], FP8, tag="e", name="e")
        nc.scalar.activation(e[:], s4[:], AF.Exp, bias=lnb_t[:])
        acc = acc_of[(h, ib)]
        lhs = _slab3(v8[:], h * (J16 * 80) + (2 * u) * 80, 80, 65)
        rhs = _slab3(e[:], 0, 512, 512)
        nc.tensor.matmul(acc[0:65, :], lhs, rhs, start=(u == 0),
                         stop=(u == NU - 1), perf_mode=DR)

    def normalize(h, ib):
        acc = acc_of.pop((h, ib))
        rec = spool.tile([1, 512], F32, tag="rec", name="rec")
        nc.vector.reciprocal(rec[:], acc[64:65, :])
        rb = spool.tile([64, 512], F32, tag="recb", name="rb")
        nc.gpsimd.partition_broadcast(rb[:], rec[:])
        p, hh = divmod(h, 2)
        col = p * LOCAL + ib * 512
        nc.vector.tensor_mul(at[ts(hh, 64), col:col + 512],
                             acc[0:64, :], rb[:])

    def head(h, pre_map=None):
        """Full attention for head h. pre_map[(ib, u)] = emitters that must
        run BEFORE unit (ib, u) (dependencies of that or later units)."""
        pre_map = pre_map or {}
        for ib in range(IBL):
            acc_of[(h, ib)] = psum.tile([128, 512], F32, tag="acc",
                                        name="acc")
        for ib in range(IBL):
            for u in range(NU):
                for fn in pre_map.get((ib, u), ()):
                    fn()
                unit(h, ib, u)
            normalize(h, ib)

    def wo_block(isub):
        """Output projection + residual + LayerNorm for 128 tokens."""
        ib, t = divmod(isub, 4)
        x = xpool.tile([128, D_MODEL], F32, tag="x", name="x")
        for dm in range(2):
            ops = psum.tile([128, 512], F32, tag="proj", name="ops")
            for qp in range(4):
                lhs = _slab3(at[:], (2 * qp) * LOCAL + ib * 512 + t * 128,
                             LOCAL, 128)
                rhs = _slab3(wo_sb[:], (2 * qp) * 1024 + dm * 512,
                             1024, 512)
                nc.tensor.matmul(ops[:], lhs, rhs, start=(qp == 0),
                                 stop=(qp == 3), perf_mode=DR)
            nc.vector.tensor_add(x[:, ts(dm, 512)], ops[:],
                                 hbres[isub][:, ts(dm, 512)])
        stats = spool.tile([128, 2, 6], F32, tag="bnst", name="st")
        mv = spool.tile([128, 2], F32, tag="bnmv", name="mv")
        for gg in range(2):
            nc.vector.bn_stats(stats[:, gg, :], x[:, ts(gg, 512)])
        nc.vector.bn_aggr(mv[:], stats[:])
        rstd = spool.tile([128, 1], F32, tag="rstd", name="rstd")
        nc.scalar.activation(rstd[:], mv[:, 1:2], AF.Ln, bias=eps_t[:])
        nc.scalar.activation(rstd[:], rstd[:], AF.Exp, scale=-0.5)
        nc.vector.tensor_scalar(
            x[:], x[:], mv[:, 0:1], rstd[:],
            op0=mybir.AluOpType.subtract, op1=mybir.AluOpType.mult)
        if affine:
            nc.vector.tensor_mul(x[:], x[:], gamma_b[:])
            nc.vector.tensor_add(x[:], x[:], beta_b[:])
        nc.sync.dma_start(out[ts(isub, 128), :], x[:])

    # ---- schedule ----
    def V(j16):
        return lambda: v_tile(j16)

    def K(p, jb):
        return lambda: kq_tile(p, jb, False)

    def Q(p, ib):
        return lambda: kq_tile(p, ib, True)

    # Lead-in: minimum work before the first exp can fire.
    kq_tile(0, 0, False)
    kq_tile(0, 0, True)

    # Head 0 carries the rest of pair 0's K, its own V chunks, and the
    # Q i-block-1 tile, each placed just before the unit that needs it
    # (scores unit u needs kt j-block u//2; PV unit u needs v chunks
    # 2u, 2u+1).
    h0_pre = {
        (0, 0): [V(0), V(1)],
        (0, 1): [V(2), V(3), K(0, 1)],
        (0, 2): [V(4), V(5)],
        (0, 3): [V(6), V(7), K(0, 2)],
        (0, 4): [V(8), V(9)],
        (0, 5): [V(10), V(11), K(0, 3)],
        (0, 6): [V(12), V(13)],
        (0, 7): [V(14), V(15)],
        (1, 0): [Q(0, 1)],
    }
    head(0, h0_pre)

    # Pair p+1's 6 projection tiles (4 K + 2 Q) are emitted during pair
    # p's heads.  Head 1 carries all 6 pair-1 tiles (head 0's slots are
    # taken by V and pair-0 work); afterwards each head carries 3.
    for h in range(1, N_HEAD):
        p = h // 2
        pre_map = {}
        if p < N_PAIR - 1:
            nxt = ([K(p + 1, jb) for jb in range(JB)]
                   + [Q(p + 1, ib) for ib in range(IBL)])
            if h == 1:
                mine, step = nxt, 2
            else:
                lo = 0 if h % 2 == 0 else 3
                mine, step = nxt[lo:lo + 3], 5
            for idx, fn in enumerate(mine):
                ib, u = divmod(step * idx, NU)
                pre_map.setdefault((ib, u), []).append(fn)
        head(h, pre_map)
    for isub in range(ISUB):
        wo_block(isub)


_program_cache = {}


def _get_program(affine=False):
    key = ("nc", affine)
    if key not in _program_cache:
        _program_cache[key] = build_program(affine)
    return _program_cache[key]


def _chunk_cols(w):
    """[1024, 1024] -> [128, 8*1024] with col c*1024+m = w[128c+p, m]."""
    return np.ascontiguousarray(
        w.reshape(CC, 128, D_MODEL).transpose(1, 0, 2).reshape(128, -1))


def _h_layout(hp, dt):
    """h_perm [2048, 1024] -> [128, jb(4) x c(8) x 512] in dtype dt."""
    a = hp.astype(dt)
    # [jb, t', c, p] -> [p, jb, c, t']
    a = a.reshape(JB, 512, CC, 128).transpose(3, 0, 2, 1)
    return np.ascontiguousarray(a.reshape(128, -1))


def _shard_inputs(h, Wq, Wkv, Wo, gamma, beta):
    h = np.asarray(h, np.float32)
    Wq = np.asarray(Wq, np.float32)
    Wkv = np.asarray(Wkv, np.float32)
    Wo = np.asarray(Wo, np.float32)
    gamma = np.asarray(gamma, np.float32)
    beta = np.asarray(beta, np.float32)

    # scores DoubleRow contracts the same slab twice -> fold an extra
    # 1/2 into the Wq scale
    scale = 0.5 / np.sqrt(D_HEAD)
    Wk = Wkv[:, :N_HEAD * D_HEAD]
    Wv = Wkv[:, N_HEAD * D_HEAD:]
    wq16 = _chunk_cols(Wq * scale).astype(ml_dtypes.bfloat16)
    wk16 = _chunk_cols(Wk).astype(ml_dtypes.bfloat16)
    wv8 = _chunk_cols(Wv).astype(ml_dtypes.float8_e4m3)
    wo8 = _chunk_cols(Wo).astype(ml_dtypes.float8_e4m3)

    in_maps = []
    for core in range(N_CORES):
        b, r = divmod(core, 2)
        hb_full = h[:, b, :]
        if r == 0:
            hp = hb_full
        else:
            hp = np.concatenate([hb_full[LOCAL:], hb_full[:LOCAL]], axis=0)
        in_maps.append({
            "hbt16": _h_layout(hp, ml_dtypes.bfloat16),
            "hbt8": _h_layout(hp, ml_dtypes.float8_e4m3),
            "wq16": wq16, "wk16": wk16, "wv8": wv8, "wo8": wo8,
            "hb": np.ascontiguousarray(hp[:LOCAL]),
            "gamma": gamma, "beta": beta,
        })
    return in_maps


def kernel(h, Wq, Wkv, Wo, gamma, beta, _trace=False):
    gamma = np.asarray(gamma, np.float32)
    beta = np.asarray(beta, np.float32)
    affine = not (np.all(gamma == 1.0) and np.all(beta == 0.0))
    nc = _get_program(affine)
    in_maps = _shard_inputs(h, Wq, Wkv, Wo, gamma, beta)
    res = run_bass_kernel_spmd(nc, in_maps, list(range(N_CORES)),
                               trace=_trace)
    if _trace:
        kernel.last_results = res

    out = np.empty((SEQ, BATCH, D_MODEL), np.float32)
    for core in range(N_CORES):
        b, r = divmod(core, 2)
        out[r * LOCAL:(r + 1) * LOCAL, b, :] = res.results[core]["out"]
    return out


# revision 12
# speedup vs baseline: 1.3721x; 1.0936x over previous
"""Trainium2 Bass kernel for nn_MultiHeadAttn_80126909874682.

Full MHA layer: QKV projection -> 16-head attention (seq 2048) -> output
projection -> residual -> LayerNorm, over h [2048, 4, 1024] fp32.

Sharding (8 NeuronCores, zero collectives):
  core c -> batch b = c // 2, token-half r = c % 2.
  Each core computes K/V for all 2048 tokens of its batch (all 16 heads)
  and Q / attention / output projection / LayerNorm for its 1024 local
  tokens only.  The per-core inputs are permuted so the core's local
  tokens come first; attention is invariant to the j-permutation of K/V.

v3 (fp8 DoubleRow): the attention-side matmuls (V projection, QK^T
scores, PV, output projection) run in fp8e4 with the DoubleRow perf
mode (two k-tile slabs contracted per instruction at 0.5 cycles/row).
Q/K projections stay bf16 (fp8 there dominates the output error).  The
scores matmul has only a 64-deep contraction, so its two DoubleRow
slabs alias the same data via stride-0 APs with the 2x folded into the
host-side Wq scale.  Exp runs on the ACT engine with a -ln(16) bias
(keeps e below the fp8e4 max); a ones-column appended to V makes the PV
matmul also emit the softmax denominators.  PSUM: 2x[128,1024] score
tiles + 2x[128,512] PV accumulators + 2x[128,512] projection tiles = 8
banks exactly.  ACT (256 exps) is the critical path; everything else
(copies, normalize, LayerNorm) lives on DVE/Pool/SP.
"""

import os
import sys

os.environ.setdefault("JAX_PLATFORMS", "axon")
sys.path.insert(0, "/opt/trn_rl_repo")

import numpy as np
import ml_dtypes

import concourse.bass as bass
import concourse.tile as tile
from concourse import bacc, mybir
from concourse.bass import ts
from concourse.bass_utils import run_bass_kernel_spmd

N_HEAD = 16
D_MODEL = 1024
D_HEAD = 64
SEQ = 2048
BATCH = 4
EPS = 1e-5
N_CORES = 8

LOCAL = SEQ // 2            # tokens owned per core (1024)
CC = D_MODEL // 128         # dmodel contraction chunks (8)
N_PAIR = N_HEAD // 2        # head pairs (8)
JB = SEQ // 512             # 512-token j blocks (4)
J16 = SEQ // 128            # 128-token j chunks (16)
NU = J16 // 2               # jc pairs per (head, iblock) unit (8)
IBL = LOCAL // 512          # local 512-token i blocks (2)
ISUB = LOCAL // 128         # local 128-token i sub tiles (8)
LN16 = float(np.log(16.0))

F32 = mybir.dt.float32
BF16 = mybir.dt.bfloat16
FP8 = mybir.dt.float8e4
AF = mybir.ActivationFunctionType
DR = mybir.MatmulPerfMode.DoubleRow


def _slab3(ap2, col0, slab_stride, n):
    """[P, n] view at col0 with an extra middle slab dim [slab_stride, 2]."""
    a = ap2[:, col0:col0 + 1]
    return bass.AP(
        tensor=a.tensor, offset=a.offset,
        ap=[list(a.ap[0]), [slab_stride, 2], [1, n]],
    )


def build_program(affine):
    nc = bacc.Bacc()

    hbt16 = nc.declare_dram_parameter("hbt16", [128, JB * CC * 512], BF16,
                                      isOutput=False)
    hbt8 = nc.declare_dram_parameter("hbt8", [128, JB * CC * 512], FP8,
                                     isOutput=False)
    wq16 = nc.declare_dram_parameter("wq16", [128, CC * D_MODEL], BF16,
                                     isOutput=False)
    wk16 = nc.declare_dram_parameter("wk16", [128, CC * D_MODEL], BF16,
                                     isOutput=False)
    wv8 = nc.declare_dram_parameter("wv8", [128, CC * D_MODEL], FP8,
                                    isOutput=False)
    wo8 = nc.declare_dram_parameter("wo8", [128, CC * D_MODEL], FP8,
                                    isOutput=False)
    hb = nc.declare_dram_parameter("hb", [LOCAL, D_MODEL], F32, isOutput=False)
    gamma = nc.declare_dram_parameter("gamma", [D_MODEL], F32, isOutput=False)
    beta = nc.declare_dram_parameter("beta", [D_MODEL], F32, isOutput=False)
    out = nc.declare_dram_parameter("out", [LOCAL, D_MODEL], F32,
                                    isOutput=True)

    with tile.TileContext(nc) as tc:
        with (
            tc.tile_pool(name="consts", bufs=1) as consts,
            tc.tile_pool(name="weights", bufs=1) as wpool,
            tc.tile_pool(name="hbt", bufs=1) as hpool,
            tc.tile_pool(name="ktq", bufs=1) as kpool,
            tc.tile_pool(name="vsb", bufs=1) as vpool,
            tc.tile_pool(name="attn", bufs=1) as apool,
            tc.tile_pool(name="exp", bufs=4) as epool,
            tc.tile_pool(name="small", bufs=3) as spool,
            tc.tile_pool(name="xstage", bufs=2) as xpool,
            tc.tile_pool(name="hbres", bufs=2) as rpool,
            tc.tile_pool(name="psum", bufs=2, space="PSUM") as psum,
        ):
            _emit(nc, hbt16, hbt8, wq16, wk16, wv8, wo8, hb, gamma, beta,
                  out, consts, wpool, hpool, kpool, vpool, apool, epool,
                  spool, xpool, rpool, psum, affine)

    nc.finalize()
    return nc


def _emit(nc, hbt16, hbt8, wq16, wk16, wv8, wo8, hb, gamma, beta, out,
          consts, wpool, hpool, kpool, vpool, apool, epool, spool, xpool,
          rpool, psum, affine):
    # ---- SBUF tiles ----
    eps_t = consts.tile([128, 1], F32)
    nc.vector.memset(eps_t[:], EPS)
    lnb_t = consts.tile([128, 1], F32)
    nc.vector.memset(lnb_t[:], -LN16)
    if affine:
        gamma_b = consts.tile([128, D_MODEL], F32)
        beta_b = consts.tile([128, D_MODEL], F32)
        g_ap, b_ap = gamma.ap(), beta.ap()
        nc.gpsimd.dma_start(
            out=gamma_b[:],
            in_=bass.AP(tensor=g_ap.tensor, offset=g_ap.offset,
                        ap=[[0, 128], [1, D_MODEL]]))
        nc.gpsimd.dma_start(
            out=beta_b[:],
            in_=bass.AP(tensor=b_ap.tensor, offset=b_ap.offset,
                        ap=[[0, 128], [1, D_MODEL]]))

    wk_sb = wpool.tile([128, CC * D_MODEL], BF16, name="wk")
    wq_sb = wpool.tile([128, CC * D_MODEL], BF16, name="wq")
    wv_sb = wpool.tile([128, CC * D_MODEL], FP8, name="wv")
    wo_sb = wpool.tile([128, CC * D_MODEL], FP8, name="wo")
    ht16 = hpool.tile([128, JB * CC * 512], BF16, name="ht16")
    ht8 = hpool.tile([128, JB * CC * 512], FP8, name="ht8")
    # kt: per pair [128p = 2 heads x 64 d] x [pair(8) x j(2048)] fp8
    kt = kpool.tile([128, N_PAIR * SEQ], FP8, name="kt")
    qt = kpool.tile([128, N_PAIR * LOCAL], FP8, name="qt")
    # v8: per head 16 j-chunks x (64 dims + ones col + 15 pad)
    # (DoubleRow slab strides must be multiples of 16 elements)
    v8 = vpool.tile([128, N_HEAD * J16 * 80], FP8, name="v8")
    # at: [128p = pair dims] x [pair(8) x ib(2) x i(512)] fp8
    at = apool.tile([128, 8 * LOCAL], FP8, name="at")

    # ones columns of v8 (col h*1280 + j*80 + 64)
    for n in range(N_HEAD):
        base = n * (J16 * 80) + 64
        a = v8[:, base:base + 1]
        nc.gpsimd.memset(
            bass.AP(tensor=a.tensor, offset=a.offset,
                    ap=[list(a.ap[0]), [80, J16], [1, 1]]), 1.0)

    # ---- DMA loads ----
    # Single HWDGE (sync) queue, priority order: the DMA engines serialize
    # transfers, so the lead-in critical path (ht16 jb0 -> wk -> wq) goes
    # first; per-chunk weight loads let the projection matmuls accumulate
    # as chunks land.
    nc.sync.dma_start(ht16[:, ts(0, 4096)], hbt16[:, ts(0, 4096)])
    for c in range(CC):
        nc.sync.dma_start(wk_sb[:, ts(c, 1024)], wk16[:, ts(c, 1024)])
    for c in range(CC):
        nc.sync.dma_start(wq_sb[:, ts(c, 1024)], wq16[:, ts(c, 1024)])
    nc.sync.dma_start(ht8[:, ts(0, 4096)], hbt8[:, ts(0, 4096)])
    nc.sync.dma_start(wv_sb[:], wv8[:, :])
    for jb in range(1, JB):
        nc.sync.dma_start(ht16[:, ts(jb, 4096)], hbt16[:, ts(jb, 4096)])
        nc.sync.dma_start(ht8[:, ts(jb, 4096)], hbt8[:, ts(jb, 4096)])
    nc.sync.dma_start(wo_sb[:], wo8[:, :])
    hbres = [rpool.tile([128, D_MODEL], F32, tag=f"hbres{i % 4}",
                        name=f"hbres{i}") for i in range(ISUB)]
    for i in range(ISUB):
        nc.sync.dma_start(hbres[i][:], hb[ts(i, 128), :])

    # ---- emission helpers ----
    def kq_tile(p, jb, is_q):
        """One [128, 512] projection tile of K^T or Q^T (bf16 matmuls).
        Output partitions = (2 heads of pair p) x 64 dims."""
        w, dst, blk = (wq_sb, qt, LOCAL) if is_q else (wk_sb, kt, SEQ)
        ps = psum.tile([128, 512], F32, tag="proj", name="kqps")
        for c in range(CC):
            nc.tensor.matmul(
                ps[:], w[:, c * 1024 + p * 128: c * 1024 + (p + 1) * 128],
                ht16[:, jb * 4096 + c * 512: jb * 4096 + (c + 1) * 512],
                start=(c == 0), stop=(c == CC - 1),
            )
        nc.vector.tensor_copy(dst[:, p * blk + jb * 512:
                                  p * blk + jb * 512 + 512], ps[:])

    def v_tile(j16):
        """V projection for one 128-token j-chunk, all 16 heads (fp8 DR)."""
        jb, t0 = divmod(j16, 4)
        for half in range(2):
            ps = psum.tile([128, 512], F32, tag="proj", name="vps")
            for cp in range(CC // 2):
                lhs = _slab3(ht8[:], jb * 4096 + (2 * cp) * 512 + t0 * 128,
                             512, 128)
                rhs = _slab3(wv_sb[:], (2 * cp) * 1024 + half * 512,
                             1024, 512)
                nc.tensor.matmul(ps[:], lhs, rhs, start=(cp == 0),
                                 stop=(cp == CC // 2 - 1), perf_mode=DR)
            # strided scatter into v8: head n = 8*half+k gets cols
            # n*1280 + j16*80 .. +64
            base = (8 * half) * (J16 * 80) + j16 * 80
            a = v8[:, base:base + 1]
            dst = bass.AP(tensor=a.tensor, offset=a.offset,
                          ap=[list(a.ap[0]), [J16 * 80, 8], [1, 64]])
            nc.vector.tensor_copy(dst, ps[:])

    acc_of = {}

    def unit(h, ib, u):
        """Scores + exp + PV for head h, i-block ib, jc-pair u."""
        p, hh = divmod(h, 2)
        s4 = psum.tile([128, 1024], F32, tag="s4", name="s4")
        for uu in range(2):
            jc = 2 * u + uu
            lhs = _slab3(kt[ts(hh, 64), :], p * SEQ + jc * 128, 0, 128)
            rhs = _slab3(qt[ts(hh, 64), :], p * LOCAL + ib * 512, 0, 512)
            nc.tensor.matmul(s4[:, ts(uu, 512)], lhs, rhs,
                             start=True, stop=True, perf_mode=DR)
        e = epool.tile([128, 1024], FP8, tag="e", name="e")
        nc.scalar.activation(e[:], s4[:], AF.Exp, bias=lnb_t[:])
        acc = acc_of[(h, ib)]
        lhs = _slab3(v8[:], h * (J16 * 80) + (2 * u) * 80, 80, 65)
        rhs = _slab3(e[:], 0, 512, 512)
        nc.tensor.matmul(acc[0:65, :], lhs, rhs, start=(u == 0),
                         stop=(u == NU - 1), perf_mode=DR)

    def normalize(h, ib):
        acc = acc_of.pop((h, ib))
        rec = spool.tile([1, 512], F32, tag="rec", name="rec")
        nc.vector.reciprocal(rec[:], acc[64:65, :])
        rb = spool.tile([64, 512], F32, tag="recb", name="rb")
        nc.gpsimd.partition_broadcast(rb[:], rec[:])
        p, hh = divmod(h, 2)
        col = p * LOCAL + ib * 512
        nc.vector.tensor_mul(at[ts(hh, 64), col:col + 512],
                             acc[0:64, :], rb[:])

    def pair(p, pre_map=None, post_map=None):
        """Attention for heads 2p, 2p+1 with their units interleaved (the
        two exps per u-slot double the pipeline window for carried work).
        pre_map[(ib, u)] = emitters run BEFORE that u-slot; post_map
        likewise after the slot."""
        pre_map = pre_map or {}
        post_map = post_map or {}
        h0, h1 = 2 * p, 2 * p + 1
        for ib in range(IBL):
            acc_of[(h0, ib)] = psum.tile([128, 512], F32, tag="acc",
                                         name="acc")
            acc_of[(h1, ib)] = psum.tile([128, 512], F32, tag="acc",
                                         name="acc")
            for u in range(NU):
                for fn in pre_map.get((ib, u), ()):
                    fn()
                unit(h0, ib, u)
                unit(h1, ib, u)
                for fn in post_map.get((ib, u), ()):
                    fn()
            normalize(h0, ib)
            normalize(h1, ib)

    def wo_block(isub):
        """Output projection + residual + LayerNorm for 128 tokens."""
        ib, t = divmod(isub, 4)
        x = xpool.tile([128, D_MODEL], F32, tag="x", name="x")
        for dm in range(2):
            ops = psum.tile([128, 512], F32, tag="proj", name="ops")
            for qp in range(4):
                lhs = _slab3(at[:], (2 * qp) * LOCAL + ib * 512 + t * 128,
                             LOCAL, 128)
                rhs = _slab3(wo_sb[:], (2 * qp) * 1024 + dm * 512,
                             1024, 512)
                nc.tensor.matmul(ops[:], lhs, rhs, start=(qp == 0),
                                 stop=(qp == 3), perf_mode=DR)
            nc.vector.tensor_add(x[:, ts(dm, 512)], ops[:],
                                 hbres[isub][:, ts(dm, 512)])
        stats = spool.tile([128, 2, 6], F32, tag="bnst", name="st")
        mv = spool.tile([128, 2], F32, tag="bnmv", name="mv")
        for gg in range(2):
            nc.vector.bn_stats(stats[:, gg, :], x[:, ts(gg, 512)])
        nc.vector.bn_aggr(mv[:], stats[:])
        rstd = spool.tile([128, 1], F32, tag="rstd", name="rstd")
        nc.scalar.activation(rstd[:], mv[:, 1:2], AF.Ln, bias=eps_t[:])
        nc.scalar.activation(rstd[:], rstd[:], AF.Exp, scale=-0.5)
        nc.vector.tensor_scalar(
            x[:], x[:], mv[:, 0:1], rstd[:],
            op0=mybir.AluOpType.subtract, op1=mybir.AluOpType.mult)
        if affine:
            nc.vector.tensor_mul(x[:], x[:], gamma_b[:])
            nc.vector.tensor_add(x[:], x[:], beta_b[:])
        nc.sync.dma_start(out[ts(isub, 128), :], x[:])

    # ---- schedule ----
    def V(j16):
        return lambda: v_tile(j16)

    def K(p, jb):
        return lambda: kq_tile(p, jb, False)

    def Q(p, ib):
        return lambda: kq_tile(p, ib, True)

    # Lead-in: minimum work before the first exp can fire.
    kq_tile(0, 0, False)
    kq_tile(0, 0, True)

    # Pair 0 carries its own V chunks / remaining K / Q(ib1) plus pair
    # 1's 6 tiles; V(2u), V(2u+1) must land before u-slot u (PV dep),
    # kt j-block b before u-slot 2b (scores dep).
    p0_pre = {
        (0, 0): [V(0), V(1)],
        (0, 1): [V(2), V(3), K(0, 1)],
        (0, 2): [V(4), V(5)],
        (0, 3): [V(6), V(7), K(0, 2)],
        (0, 4): [V(8), V(9)],
        (0, 5): [V(10), V(11), K(0, 3)],
        (0, 6): [V(12), V(13)],
        (0, 7): [V(14), V(15), Q(0, 1)],
        (1, 0): [K(1, 0)],
        (1, 1): [K(1, 1)],
        (1, 2): [K(1, 2)],
        (1, 4): [K(1, 3)],
        (1, 5): [Q(1, 0)],
        (1, 6): [Q(1, 1)],
    }
    pair(0, p0_pre)

    # Pairs 1..6 carry pair p+1's 6 tiles spread over their 16 u-slots;
    # the last pair interleaves the first half of the output projection
    # blocks into its ib-1 phase (their at-deps complete at ib-0's end).
    for p in range(1, N_PAIR):
        pre_map = {}
        post_map = {}
        if p < N_PAIR - 1:
            nxt = ([K(p + 1, jb) for jb in range(JB)]
                   + [Q(p + 1, ib) for ib in range(IBL)])
            for idx, fn in enumerate(nxt):
                ib, u = divmod(2 * idx + 2, NU)
                pre_map.setdefault((ib, u), []).append(fn)
        else:
            for isub in range(4):
                post_map.setdefault((1, 2 * isub), []).append(
                    lambda isub=isub: wo_block(isub))
        pair(p, pre_map, post_map)
    for isub in range(4, ISUB):
        wo_block(isub)


_program_cache = {}


def _get_program(affine=False):
    key = ("nc", affine)
    if key not in _program_cache:
        _program_cache[key] = build_program(affine)
    return _program_cache[key]


def _chunk_cols(w):
    """[1024, 1024] -> [128, 8*1024] with col c*1024+m = w[128c+p, m]."""
    return np.ascontiguousarray(
        w.reshape(CC, 128, D_MODEL).transpose(1, 0, 2).reshape(128, -1))


def _h_layout(hp, dt):
    """h_perm [2048, 1024] -> [128, jb(4) x c(8) x 512] in dtype dt."""
    a = hp.astype(dt)
    # [jb, t', c, p] -> [p, jb, c, t']
    a = a.reshape(JB, 512, CC, 128).transpose(3, 0, 2, 1)
    return np.ascontiguousarray(a.reshape(128, -1))


def _shard_inputs(h, Wq, Wkv, Wo, gamma, beta):
    h = np.asarray(h, np.float32)
    Wq = np.asarray(Wq, np.float32)
    Wkv = np.asarray(Wkv, np.float32)
    Wo = np.asarray(Wo, np.float32)
    gamma = np.asarray(gamma, np.float32)
    beta = np.asarray(beta, np.float32)

    # scores DoubleRow contracts the same slab twice -> fold an extra
    # 1/2 into the Wq scale
    scale = 0.5 / np.sqrt(D_HEAD)
    Wk = Wkv[:, :N_HEAD * D_HEAD]
    Wv = Wkv[:, N_HEAD * D_HEAD:]
    wq16 = _chunk_cols(Wq * scale).astype(ml_dtypes.bfloat16)
    wk16 = _chunk_cols(Wk).astype(ml_dtypes.bfloat16)
    wv8 = _chunk_cols(Wv).astype(ml_dtypes.float8_e4m3)
    wo8 = _chunk_cols(Wo).astype(ml_dtypes.float8_e4m3)

    in_maps = []
    for core in range(N_CORES):
        b, r = divmod(core, 2)
        hb_full = h[:, b, :]
        if r == 0:
            hp = hb_full
        else:
            hp = np.concatenate([hb_full[LOCAL:], hb_full[:LOCAL]], axis=0)
        in_maps.append({
            "hbt16": _h_layout(hp, ml_dtypes.bfloat16),
            "hbt8": _h_layout(hp, ml_dtypes.float8_e4m3),
            "wq16": wq16, "wk16": wk16, "wv8": wv8, "wo8": wo8,
            "hb": np.ascontiguousarray(hp[:LOCAL]),
            "gamma": gamma, "beta": beta,
        })
    return in_maps


def kernel(h, Wq, Wkv, Wo, gamma, beta, _trace=False):
    gamma = np.asarray(gamma, np.float32)
    beta = np.asarray(beta, np.float32)
    affine = not (np.all(gamma == 1.0) and np.all(beta == 0.0))
    nc = _get_program(affine)
    in_maps = _shard_inputs(h, Wq, Wkv, Wo, gamma, beta)
    res = run_bass_kernel_spmd(nc, in_maps, list(range(N_CORES)),
                               trace=_trace)
    if _trace:
        kernel.last_results = res

    out = np.empty((SEQ, BATCH, D_MODEL), np.float32)
    for core in range(N_CORES):
        b, r = divmod(core, 2)
        out[r * LOCAL:(r + 1) * LOCAL, b, :] = res.results[core]["out"]
    return out


# revision 15
# speedup vs baseline: 1.3927x; 1.0150x over previous
"""Trainium2 Bass kernel for nn_MultiHeadAttn_80126909874682.

Full MHA layer: QKV projection -> 16-head attention (seq 2048) -> output
projection -> residual -> LayerNorm, over h [2048, 4, 1024] fp32.

Sharding (8 NeuronCores, zero collectives):
  core c -> batch b = c // 2, token-half r = c % 2.
  Each core computes K/V for all 2048 tokens of its batch (all 16 heads)
  and Q / attention / output projection / LayerNorm for its 1024 local
  tokens only.  The per-core inputs are permuted so the core's local
  tokens come first; attention is invariant to the j-permutation of K/V.

v3 (fp8 DoubleRow): the attention-side matmuls (V projection, QK^T
scores, PV, output projection) run in fp8e4 with the DoubleRow perf
mode (two k-tile slabs contracted per instruction at 0.5 cycles/row).
Q/K projections stay bf16 (fp8 there dominates the output error).  The
scores matmul has only a 64-deep contraction, so its two DoubleRow
slabs alias the same data via stride-0 APs with the 2x folded into the
host-side Wq scale.  Exp runs on the ACT engine with a -ln(16) bias
(keeps e below the fp8e4 max); a ones-column appended to V makes the PV
matmul also emit the softmax denominators.  PSUM: 2x[128,1024] score
tiles + 2x[128,512] PV accumulators + 2x[128,512] projection tiles = 8
banks exactly.  ACT (256 exps) is the critical path; everything else
(copies, normalize, LayerNorm) lives on DVE/Pool/SP.
"""

import os
import sys

os.environ.setdefault("JAX_PLATFORMS", "axon")
sys.path.insert(0, "/opt/trn_rl_repo")

import numpy as np
import ml_dtypes

import concourse.bass as bass
import concourse.tile as tile
from concourse import bacc, mybir
from concourse.bass import ts
from concourse.bass_utils import run_bass_kernel_spmd

N_HEAD = 16
D_MODEL = 1024
D_HEAD = 64
SEQ = 2048
BATCH = 4
EPS = 1e-5
N_CORES = 8

LOCAL = SEQ // 2            # tokens owned per core (1024)
CC = D_MODEL // 128         # dmodel contraction chunks (8)
N_PAIR = N_HEAD // 2        # head pairs (8)
JB = SEQ // 512             # 512-token j blocks (4)
J16 = SEQ // 128            # 128-token j chunks (16)
NU = J16 // 2               # jc pairs per (head, iblock) unit (8)
IBL = LOCAL // 512          # local 512-token i blocks (2)
ISUB = LOCAL // 128         # local 128-token i sub tiles (8)
LN16 = float(np.log(16.0))

F32 = mybir.dt.float32
BF16 = mybir.dt.bfloat16
FP8 = mybir.dt.float8e4
AF = mybir.ActivationFunctionType
DR = mybir.MatmulPerfMode.DoubleRow


def _slab3(ap2, col0, slab_stride, n):
    """[P, n] view at col0 with an extra middle slab dim [slab_stride, 2]."""
    a = ap2[:, col0:col0 + 1]
    return bass.AP(
        tensor=a.tensor, offset=a.offset,
        ap=[list(a.ap[0]), [slab_stride, 2], [1, n]],
    )


def build_program(affine):
    nc = bacc.Bacc()

    hbt16 = nc.declare_dram_parameter("hbt16", [128, JB * CC * 512], BF16,
                                      isOutput=False)
    hbt8 = nc.declare_dram_parameter("hbt8", [128, JB * CC * 512], FP8,
                                     isOutput=False)
    wq16 = nc.declare_dram_parameter("wq16", [128, CC * D_MODEL], BF16,
                                     isOutput=False)
    wk16 = nc.declare_dram_parameter("wk16", [128, CC * D_MODEL], BF16,
                                     isOutput=False)
    wv8 = nc.declare_dram_parameter("wv8", [128, CC * D_MODEL], FP8,
                                    isOutput=False)
    wo8 = nc.declare_dram_parameter("wo8", [128, CC * D_MODEL], FP8,
                                    isOutput=False)
    hb = nc.declare_dram_parameter("hb", [LOCAL, D_MODEL], BF16,
                                   isOutput=False)
    gamma = nc.declare_dram_parameter("gamma", [D_MODEL], F32, isOutput=False)
    beta = nc.declare_dram_parameter("beta", [D_MODEL], F32, isOutput=False)
    out = nc.declare_dram_parameter("out", [LOCAL, D_MODEL], F32,
                                    isOutput=True)

    with tile.TileContext(nc) as tc:
        with (
            tc.tile_pool(name="consts", bufs=1) as consts,
            tc.tile_pool(name="weights", bufs=1) as wpool,
            tc.tile_pool(name="hbt", bufs=1) as hpool,
            tc.tile_pool(name="ktq", bufs=1) as kpool,
            tc.tile_pool(name="vsb", bufs=1) as vpool,
            tc.tile_pool(name="attn", bufs=1) as apool,
            tc.tile_pool(name="exp", bufs=4) as epool,
            tc.tile_pool(name="small", bufs=3) as spool,
            tc.tile_pool(name="xstage", bufs=1) as xpool,
            tc.tile_pool(name="hbres", bufs=2) as rpool,
            tc.tile_pool(name="psum", bufs=2, space="PSUM") as psum,
        ):
            _emit(nc, hbt16, hbt8, wq16, wk16, wv8, wo8, hb, gamma, beta,
                  out, consts, wpool, hpool, kpool, vpool, apool, epool,
                  spool, xpool, rpool, psum, affine)

    nc.finalize()
    return nc


def _emit(nc, hbt16, hbt8, wq16, wk16, wv8, wo8, hb, gamma, beta, out,
          consts, wpool, hpool, kpool, vpool, apool, epool, spool, xpool,
          rpool, psum, affine):
    # ---- SBUF tiles ----
    eps_t = consts.tile([128, 1], F32)
    nc.vector.memset(eps_t[:], EPS)
    lnb_t = consts.tile([128, 1], F32)
    nc.vector.memset(lnb_t[:], -LN16)
    if affine:
        gamma_b = consts.tile([128, D_MODEL], F32)
        beta_b = consts.tile([128, D_MODEL], F32)
        g_ap, b_ap = gamma.ap(), beta.ap()
        nc.gpsimd.dma_start(
            out=gamma_b[:],
            in_=bass.AP(tensor=g_ap.tensor, offset=g_ap.offset,
                        ap=[[0, 128], [1, D_MODEL]]))
        nc.gpsimd.dma_start(
            out=beta_b[:],
            in_=bass.AP(tensor=b_ap.tensor, offset=b_ap.offset,
                        ap=[[0, 128], [1, D_MODEL]]))

    wk_sb = wpool.tile([128, CC * D_MODEL], BF16, name="wk")
    wq_sb = wpool.tile([128, CC * D_MODEL], BF16, name="wq")
    wv_sb = wpool.tile([128, CC * D_MODEL], FP8, name="wv")
    wo_sb = wpool.tile([128, CC * D_MODEL], FP8, name="wo")
    ht16 = hpool.tile([128, JB * CC * 512], BF16, name="ht16")
    ht8 = hpool.tile([128, JB * CC * 512], FP8, name="ht8")
    # kt: per pair [128p = 2 heads x 64 d] x [pair(8) x j(2048)] fp8
    kt = kpool.tile([128, N_PAIR * SEQ], FP8, name="kt")
    qt = kpool.tile([128, N_PAIR * LOCAL], FP8, name="qt")
    # v8: per head 16 j-chunks x (64 dims + ones col + 15 pad)
    # (DoubleRow slab strides must be multiples of 16 elements)
    v8 = vpool.tile([128, N_HEAD * J16 * 80], FP8, name="v8")
    # at: [128p = pair dims] x [pair(8) x ib(2) x i(512)] fp8
    at = apool.tile([128, 8 * LOCAL], FP8, name="at")

    # ones columns of v8 (col h*1280 + j*80 + 64)
    for n in range(N_HEAD):
        base = n * (J16 * 80) + 64
        a = v8[:, base:base + 1]
        nc.gpsimd.memset(
            bass.AP(tensor=a.tensor, offset=a.offset,
                    ap=[list(a.ap[0]), [80, J16], [1, 1]]), 1.0)

    # ---- DMA loads ----
    # Single HWDGE (sync) queue, priority order: the DMA engines serialize
    # transfers, so the lead-in critical path (ht16 jb0 -> wk -> wq) goes
    # first; per-chunk weight loads let the projection matmuls accumulate
    # as chunks land.
    nc.sync.dma_start(ht16[:, ts(0, 4096)], hbt16[:, ts(0, 4096)])
    for c in range(CC):
        nc.sync.dma_start(wk_sb[:, ts(c, 1024)], wk16[:, ts(c, 1024)])
    for c in range(CC):
        nc.sync.dma_start(wq_sb[:, ts(c, 1024)], wq16[:, ts(c, 1024)])
    nc.sync.dma_start(ht8[:, ts(0, 4096)], hbt8[:, ts(0, 4096)])
    nc.sync.dma_start(wv_sb[:], wv8[:, :])
    for jb in range(1, JB):
        nc.sync.dma_start(ht16[:, ts(jb, 4096)], hbt16[:, ts(jb, 4096)])
        nc.sync.dma_start(ht8[:, ts(jb, 4096)], hbt8[:, ts(jb, 4096)])
    nc.sync.dma_start(wo_sb[:], wo8[:, :])
    hbres = [rpool.tile([128, D_MODEL], BF16, tag=f"hbres{i % 2}",
                        name=f"hbres{i}") for i in range(ISUB)]
    for i in range(ISUB):
        nc.sync.dma_start(hbres[i][:], hb[ts(i, 128), :])

    # ---- emission helpers ----
    def kq_tile(p, jb, is_q):
        """One [128, 512] projection tile of K^T or Q^T (bf16 matmuls).
        Output partitions = (2 heads of pair p) x 64 dims."""
        w, dst, blk = (wq_sb, qt, LOCAL) if is_q else (wk_sb, kt, SEQ)
        ps = psum.tile([128, 512], F32, tag="proj", name="kqps")
        for c in range(CC):
            nc.tensor.matmul(
                ps[:], w[:, c * 1024 + p * 128: c * 1024 + (p + 1) * 128],
                ht16[:, jb * 4096 + c * 512: jb * 4096 + (c + 1) * 512],
                start=(c == 0), stop=(c == CC - 1),
            )
        nc.vector.tensor_copy(dst[:, p * blk + jb * 512:
                                  p * blk + jb * 512 + 512], ps[:])

    def v_tile(j16):
        """V projection for one 128-token j-chunk, all 16 heads (fp8 DR)."""
        jb, t0 = divmod(j16, 4)
        for half in range(2):
            ps = psum.tile([128, 512], F32, tag="proj", name="vps")
            for cp in range(CC // 2):
                lhs = _slab3(ht8[:], jb * 4096 + (2 * cp) * 512 + t0 * 128,
                             512, 128)
                rhs = _slab3(wv_sb[:], (2 * cp) * 1024 + half * 512,
                             1024, 512)
                nc.tensor.matmul(ps[:], lhs, rhs, start=(cp == 0),
                                 stop=(cp == CC // 2 - 1), perf_mode=DR)
            # strided scatter into v8: head n = 8*half+k gets cols
            # n*1280 + j16*80 .. +64
            base = (8 * half) * (J16 * 80) + j16 * 80
            a = v8[:, base:base + 1]
            dst = bass.AP(tensor=a.tensor, offset=a.offset,
                          ap=[list(a.ap[0]), [J16 * 80, 8], [1, 64]])
            nc.vector.tensor_copy(dst, ps[:])

    acc_of = {}

    def unit(h, ib, u):
        """Scores + exp + PV for head h, i-block ib, jc-pair u."""
        p, hh = divmod(h, 2)
        s4 = psum.tile([128, 1024], F32, tag="s4", name="s4")
        for uu in range(2):
            jc = 2 * u + uu
            lhs = _slab3(kt[ts(hh, 64), :], p * SEQ + jc * 128, 0, 128)
            rhs = _slab3(qt[ts(hh, 64), :], p * LOCAL + ib * 512, 0, 512)
            nc.tensor.matmul(s4[:, ts(uu, 512)], lhs, rhs,
                             start=True, stop=True, perf_mode=DR)
        e = epool.tile([128, 1024], FP8, tag="e", name="e")
        nc.scalar.activation(e[:], s4[:], AF.Exp, bias=lnb_t[:])
        acc = acc_of[(h, ib)]
        lhs = _slab3(v8[:], h * (J16 * 80) + (2 * u) * 80, 80, 65)
        rhs = _slab3(e[:], 0, 512, 512)
        nc.tensor.matmul(acc[0:65, :], lhs, rhs, start=(u == 0),
                         stop=(u == NU - 1), perf_mode=DR)

    def normalize(h, ib):
        acc = acc_of.pop((h, ib))
        rec = spool.tile([1, 512], F32, tag="rec", name="rec")
        nc.vector.reciprocal(rec[:], acc[64:65, :])
        rb = spool.tile([64, 512], F32, tag="recb", name="rb")
        nc.gpsimd.partition_broadcast(rb[:], rec[:])
        p, hh = divmod(h, 2)
        col = p * LOCAL + ib * 512
        nc.vector.tensor_mul(at[ts(hh, 64), col:col + 512],
                             acc[0:64, :], rb[:])

    def pair(p, pre_map=None, post_map=None):
        """Attention for heads 2p, 2p+1 with their units interleaved (the
        two exps per u-slot double the pipeline window for carried work).
        pre_map[(ib, u)] = emitters run BEFORE that u-slot; post_map
        likewise after the slot."""
        pre_map = pre_map or {}
        post_map = post_map or {}
        h0, h1 = 2 * p, 2 * p + 1
        for ib in range(IBL):
            acc_of[(h0, ib)] = psum.tile([128, 512], F32, tag="acc",
                                         name="acc")
            acc_of[(h1, ib)] = psum.tile([128, 512], F32, tag="acc",
                                         name="acc")
            for u in range(NU):
                for fn in pre_map.get((ib, u), ()):
                    fn()
                unit(h0, ib, u)
                unit(h1, ib, u)
                for fn in post_map.get((ib, u), ()):
                    fn()
            normalize(h0, ib)
            normalize(h1, ib)

    # Output blocks in three phases so the per-block Ln/Exp rstd pairs
    # don't thrash the ACT function table against the attention exps:
    # A) projection + residual + bn stats (no ACT), B) one batched
    # Ln/Exp over all 8 variances after the last exp, C) normalize+store.
    x_of = [None] * ISUB
    mv_of = [None] * ISUB
    vbat = consts.tile([128, ISUB], F32, name="vbat")
    rbat = consts.tile([128, ISUB], F32, name="rbat")

    def wo_a(isub):
        ib, t = divmod(isub, 4)
        x = xpool.tile([128, D_MODEL], F32, tag=f"x{isub}", name="x")
        x_of[isub] = x
        for dm in range(2):
            ops = psum.tile([128, 512], F32, tag="proj", name="ops")
            for qp in range(4):
                lhs = _slab3(at[:], (2 * qp) * LOCAL + ib * 512 + t * 128,
                             LOCAL, 128)
                rhs = _slab3(wo_sb[:], (2 * qp) * 1024 + dm * 512,
                             1024, 512)
                nc.tensor.matmul(ops[:], lhs, rhs, start=(qp == 0),
                                 stop=(qp == 3), perf_mode=DR)
            nc.vector.tensor_add(x[:, ts(dm, 512)], ops[:],
                                 hbres[isub][:, ts(dm, 512)])
        stats = spool.tile([128, 2, 6], F32, tag="bnst", name="st")
        mv = spool.tile([128, 2], F32, tag=f"bnmv{isub}", name="mv")
        mv_of[isub] = mv
        for gg in range(2):
            nc.vector.bn_stats(stats[:, gg, :], x[:, ts(gg, 512)])
        nc.vector.bn_aggr(mv[:], stats[:])
        nc.vector.tensor_copy(vbat[:, isub:isub + 1], mv[:, 1:2])

    def wo_b():
        nc.scalar.activation(rbat[:], vbat[:], AF.Ln, bias=eps_t[:])
        nc.scalar.activation(rbat[:], rbat[:], AF.Exp, scale=-0.5)

    def wo_c(isub):
        x = x_of[isub]
        nc.vector.tensor_scalar(
            x[:], x[:], mv_of[isub][:, 0:1], rbat[:, isub:isub + 1],
            op0=mybir.AluOpType.subtract, op1=mybir.AluOpType.mult)
        if affine:
            nc.vector.tensor_mul(x[:], x[:], gamma_b[:])
            nc.vector.tensor_add(x[:], x[:], beta_b[:])
        nc.sync.dma_start(out[ts(isub, 128), :], x[:])

    # ---- schedule ----
    def V(j16):
        return lambda: v_tile(j16)

    def K(p, jb):
        return lambda: kq_tile(p, jb, False)

    def Q(p, ib):
        return lambda: kq_tile(p, ib, True)

    # Lead-in: minimum work before the first exp can fire.
    kq_tile(0, 0, False)
    kq_tile(0, 0, True)

    # Each pair self-carries its own later K j-blocks (needed at u-slot
    # 2b) and Q i-block 1, plus the NEXT pair's first K/Q; pair 0 also
    # carries all 16 V chunks (V(2u), V(2u+1) before u-slot u for PV).
    p0_pre = {
        (0, 0): [V(0), V(1)],
        (0, 1): [V(2), V(3)],
        (0, 2): [V(4), V(5), K(0, 1)],
        (0, 3): [V(6), V(7)],
        (0, 4): [V(8), V(9), K(0, 2)],
        (0, 5): [V(10), V(11)],
        (0, 6): [V(12), V(13), K(0, 3)],
        (0, 7): [V(14), V(15), Q(0, 1)],
        (1, 1): [K(1, 0)],
        (1, 4): [Q(1, 0)],
    }
    pair(0, p0_pre)

    # Pairs 1..7: self-carry K jb 1-3 at u-slots 2,4,6 and Q ib1 at
    # slot 7; hand the next pair its first K/Q during ib 1.  The last
    # pair interleaves the first output-projection blocks into its ib-1
    # phase (their at-deps complete at ib-0's end).
    for p in range(1, N_PAIR):
        pre_map = {
            (0, 2): [K(p, 1)],
            (0, 4): [K(p, 2)],
            (0, 6): [K(p, 3)],
            (0, 7): [Q(p, 1)],
        }
        post_map = {}
        if p < N_PAIR - 1:
            pre_map[(1, 1)] = [K(p + 1, 0)]
            pre_map[(1, 4)] = [Q(p + 1, 0)]
        else:
            for isub in range(4):
                post_map.setdefault((1, 2 * isub), []).append(
                    lambda isub=isub: wo_a(isub))
        pair(p, pre_map, post_map)
    for isub in range(4, ISUB):
        wo_a(isub)
    wo_b()
    for isub in range(ISUB):
        wo_c(isub)


_program_cache = {}


def _get_program(affine=False):
    key = ("nc", affine)
    if key not in _program_cache:
        _program_cache[key] = build_program(affine)
    return _program_cache[key]


def _chunk_cols(w):
    """[1024, 1024] -> [128, 8*1024] with col c*1024+m = w[128c+p, m]."""
    return np.ascontiguousarray(
        w.reshape(CC, 128, D_MODEL).transpose(1, 0, 2).reshape(128, -1))


def _h_layout(hp, dt):
    """h_perm [2048, 1024] -> [128, jb(4) x c(8) x 512] in dtype dt."""
    a = hp.astype(dt)
    # [jb, t', c, p] -> [p, jb, c, t']
    a = a.reshape(JB, 512, CC, 128).transpose(3, 0, 2, 1)
    return np.ascontiguousarray(a.reshape(128, -1))


def _shard_inputs(h, Wq, Wkv, Wo, gamma, beta):
    h = np.asarray(h, np.float32)
    Wq = np.asarray(Wq, np.float32)
    Wkv = np.asarray(Wkv, np.float32)
    Wo = np.asarray(Wo, np.float32)
    gamma = np.asarray(gamma, np.float32)
    beta = np.asarray(beta, np.float32)

    # scores DoubleRow contracts the same slab twice -> fold an extra
    # 1/2 into the Wq scale
    scale = 0.5 / np.sqrt(D_HEAD)
    Wk = Wkv[:, :N_HEAD * D_HEAD]
    Wv = Wkv[:, N_HEAD * D_HEAD:]
    wq16 = _chunk_cols(Wq * scale).astype(ml_dtypes.bfloat16)
    wk16 = _chunk_cols(Wk).astype(ml_dtypes.bfloat16)
    wv8 = _chunk_cols(Wv).astype(ml_dtypes.float8_e4m3)
    wo8 = _chunk_cols(Wo).astype(ml_dtypes.float8_e4m3)

    in_maps = []
    for core in range(N_CORES):
        b, r = divmod(core, 2)
        hb_full = h[:, b, :]
        if r == 0:
            hp = hb_full
        else:
            hp = np.concatenate([hb_full[LOCAL:], hb_full[:LOCAL]], axis=0)
        in_maps.append({
            "hbt16": _h_layout(hp, ml_dtypes.bfloat16),
            "hbt8": _h_layout(hp, ml_dtypes.float8_e4m3),
            "wq16": wq16, "wk16": wk16, "wv8": wv8, "wo8": wo8,
            "hb": np.ascontiguousarray(hp[:LOCAL].astype(ml_dtypes.bfloat16)),
            "gamma": gamma, "beta": beta,
        })
    return in_maps


def kernel(h, Wq, Wkv, Wo, gamma, beta, _trace=False):
    gamma = np.asarray(gamma, np.float32)
    beta = np.asarray(beta, np.float32)
    affine = not (np.all(gamma == 1.0) and np.all(beta == 0.0))
    nc = _get_program(affine)
    in_maps = _shard_inputs(h, Wq, Wkv, Wo, gamma, beta)
    res = run_bass_kernel_spmd(nc, in_maps, list(range(N_CORES)),
                               trace=_trace)
    if _trace:
        kernel.last_results = res

    out = np.empty((SEQ, BATCH, D_MODEL), np.float32)
    for core in range(N_CORES):
        b, r = divmod(core, 2)
        out[r * LOCAL:(r + 1) * LOCAL, b, :] = res.results[core]["out"]
    return out


# revision 16
# speedup vs baseline: 1.4330x; 1.0290x over previous
"""Trainium2 Bass kernel for nn_MultiHeadAttn_80126909874682.

Full MHA layer: QKV projection -> 16-head attention (seq 2048) -> output
projection -> residual -> LayerNorm, over h [2048, 4, 1024] fp32.

Sharding (8 NeuronCores, zero collectives):
  core c -> batch b = c // 2, token-half r = c % 2.
  Each core computes K/V for all 2048 tokens of its batch (all 16 heads)
  and Q / attention / output projection / LayerNorm for its 1024 local
  tokens only.  The per-core inputs are permuted so the core's local
  tokens come first; attention is invariant to the j-permutation of K/V.

v3 (fp8 DoubleRow): the attention-side matmuls (V projection, QK^T
scores, PV, output projection) run in fp8e4 with the DoubleRow perf
mode (two k-tile slabs contracted per instruction at 0.5 cycles/row).
Q/K projections stay bf16 (fp8 there dominates the output error).  The
scores matmul has only a 64-deep contraction, so its two DoubleRow
slabs alias the same data via stride-0 APs with the 2x folded into the
host-side Wq scale.  Exp runs on the ACT engine with a -ln(16) bias
(keeps e below the fp8e4 max); a ones-column appended to V makes the PV
matmul also emit the softmax denominators.  PSUM: 2x[128,1024] score
tiles + 2x[128,512] PV accumulators + 2x[128,512] projection tiles = 8
banks exactly.  ACT (256 exps) is the critical path; everything else
(copies, normalize, LayerNorm) lives on DVE/Pool/SP.
"""

import os
import sys

os.environ.setdefault("JAX_PLATFORMS", "axon")
sys.path.insert(0, "/opt/trn_rl_repo")

import numpy as np
import ml_dtypes

import concourse.bass as bass
import concourse.tile as tile
from concourse import bacc, mybir
from concourse.bass import ts
from concourse.bass_utils import run_bass_kernel_spmd

N_HEAD = 16
D_MODEL = 1024
D_HEAD = 64
SEQ = 2048
BATCH = 4
EPS = 1e-5
N_CORES = 8

LOCAL = SEQ // 2            # tokens owned per core (1024)
CC = D_MODEL // 128         # dmodel contraction chunks (8)
N_PAIR = N_HEAD // 2        # head pairs (8)
JB = SEQ // 512             # 512-token j blocks (4)
J16 = SEQ // 128            # 128-token j chunks (16)
NU = J16 // 2               # jc pairs per (head, iblock) unit (8)
IBL = LOCAL // 512          # local 512-token i blocks (2)
ISUB = LOCAL // 128         # local 128-token i sub tiles (8)
LN16 = float(np.log(16.0))

F32 = mybir.dt.float32
BF16 = mybir.dt.bfloat16
FP8 = mybir.dt.float8e4
AF = mybir.ActivationFunctionType
DR = mybir.MatmulPerfMode.DoubleRow


def _slab3(ap2, col0, slab_stride, n):
    """[P, n] view at col0 with an extra middle slab dim [slab_stride, 2]."""
    a = ap2[:, col0:col0 + 1]
    return bass.AP(
        tensor=a.tensor, offset=a.offset,
        ap=[list(a.ap[0]), [slab_stride, 2], [1, n]],
    )


def build_program(affine):
    nc = bacc.Bacc()

    hbt16 = nc.declare_dram_parameter("hbt16", [128, JB * CC * 512], BF16,
                                      isOutput=False)
    hbt8 = nc.declare_dram_parameter("hbt8", [128, JB * CC * 512], FP8,
                                     isOutput=False)
    wq16 = nc.declare_dram_parameter("wq16", [128, CC * D_MODEL], BF16,
                                     isOutput=False)
    wk16 = nc.declare_dram_parameter("wk16", [128, CC * D_MODEL], BF16,
                                     isOutput=False)
    wv8 = nc.declare_dram_parameter("wv8", [128, CC * D_MODEL], FP8,
                                    isOutput=False)
    wo8 = nc.declare_dram_parameter("wo8", [128, CC * D_MODEL], FP8,
                                    isOutput=False)
    hb = nc.declare_dram_parameter("hb", [LOCAL, D_MODEL], BF16,
                                   isOutput=False)
    gamma = nc.declare_dram_parameter("gamma", [D_MODEL], F32, isOutput=False)
    beta = nc.declare_dram_parameter("beta", [D_MODEL], F32, isOutput=False)
    out = nc.declare_dram_parameter("out", [LOCAL, D_MODEL], F32,
                                    isOutput=True)

    with tile.TileContext(nc) as tc:
        with (
            tc.tile_pool(name="consts", bufs=1) as consts,
            tc.tile_pool(name="weights", bufs=1) as wpool,
            tc.tile_pool(name="hbt", bufs=1) as hpool,
            tc.tile_pool(name="ktq", bufs=1) as kpool,
            tc.tile_pool(name="vsb", bufs=1) as vpool,
            tc.tile_pool(name="attn", bufs=1) as apool,
            tc.tile_pool(name="exp", bufs=4) as epool,
            tc.tile_pool(name="small", bufs=3) as spool,
            tc.tile_pool(name="xstage", bufs=1) as xpool,
            tc.tile_pool(name="hbres", bufs=2) as rpool,
            tc.tile_pool(name="psum", bufs=2, space="PSUM") as psum,
        ):
            _emit(nc, hbt16, hbt8, wq16, wk16, wv8, wo8, hb, gamma, beta,
                  out, consts, wpool, hpool, kpool, vpool, apool, epool,
                  spool, xpool, rpool, psum, affine)

    nc.finalize()
    return nc


def _emit(nc, hbt16, hbt8, wq16, wk16, wv8, wo8, hb, gamma, beta, out,
          consts, wpool, hpool, kpool, vpool, apool, epool, spool, xpool,
          rpool, psum, affine):
    # ---- SBUF tiles ----
    eps_t = consts.tile([128, 1], F32)
    nc.vector.memset(eps_t[:], EPS)
    lnb_t = consts.tile([128, 1], F32)
    nc.vector.memset(lnb_t[:], -LN16)
    if affine:
        gamma_b = consts.tile([128, D_MODEL], F32)
        beta_b = consts.tile([128, D_MODEL], F32)
        g_ap, b_ap = gamma.ap(), beta.ap()
        nc.gpsimd.dma_start(
            out=gamma_b[:],
            in_=bass.AP(tensor=g_ap.tensor, offset=g_ap.offset,
                        ap=[[0, 128], [1, D_MODEL]]))
        nc.gpsimd.dma_start(
            out=beta_b[:],
            in_=bass.AP(tensor=b_ap.tensor, offset=b_ap.offset,
                        ap=[[0, 128], [1, D_MODEL]]))

    wk_sb = wpool.tile([128, CC * D_MODEL], BF16, name="wk")
    wq_sb = wpool.tile([128, CC * D_MODEL], BF16, name="wq")
    wv_sb = wpool.tile([128, CC * D_MODEL], FP8, name="wv")
    wo_sb = wpool.tile([128, CC * D_MODEL], FP8, name="wo")
    ht16 = hpool.tile([128, JB * CC * 512], BF16, name="ht16")
    ht8 = hpool.tile([128, JB * CC * 512], FP8, name="ht8")
    # kt: per pair [128p = 2 heads x 64 d] x [pair(8) x j(2048)] fp8
    kt = kpool.tile([128, N_PAIR * SEQ], FP8, name="kt")
    qt = kpool.tile([128, N_PAIR * LOCAL], FP8, name="qt")
    # v8: per head 16 j-chunks x (64 dims + ones col + 15 pad)
    # (DoubleRow slab strides must be multiples of 16 elements)
    v8 = vpool.tile([128, N_HEAD * J16 * 80], FP8, name="v8")
    # at: per i-block [128p = pair dims] x [pair(8) x i(512)] fp8
    # (two tiles so the output projection's reads of i-block 0 don't
    # serialize behind i-block-1 normalize writes via tile-granular deps)
    at_ib = [apool.tile([128, 8 * 512], FP8, name=f"at{ib}")
             for ib in range(IBL)]

    # ones columns of v8 (col h*1280 + j*80 + 64)
    for n in range(N_HEAD):
        base = n * (J16 * 80) + 64
        a = v8[:, base:base + 1]
        nc.gpsimd.memset(
            bass.AP(tensor=a.tensor, offset=a.offset,
                    ap=[list(a.ap[0]), [80, J16], [1, 1]]), 1.0)

    # ---- DMA loads ----
    # Single HWDGE (sync) queue, priority order: the DMA engines serialize
    # transfers, so the lead-in critical path (ht16 jb0 -> wk -> wq) goes
    # first; per-chunk weight loads let the projection matmuls accumulate
    # as chunks land.
    def wv_half(half):
        a = wv_sb[:, half * 512:half * 512 + 1]
        dst = bass.AP(tensor=a.tensor, offset=a.offset,
                      ap=[list(a.ap[0]), [1024, CC], [1, 512]])
        s = wv8.ap()
        srcap = bass.AP(tensor=s.tensor, offset=s.offset + half * 512,
                        ap=[list(s.ap[0]), [1024, CC], [1, 512]])
        nc.sync.dma_start(dst, srcap)

    nc.sync.dma_start(ht16[:, ts(0, 4096)], hbt16[:, ts(0, 4096)])
    for c in range(CC):
        nc.sync.dma_start(wk_sb[:, ts(c, 1024)], wk16[:, ts(c, 1024)])
    for c in range(CC):
        nc.sync.dma_start(wq_sb[:, ts(c, 1024)], wq16[:, ts(c, 1024)])
    nc.sync.dma_start(ht8[:, ts(0, 4096)], hbt8[:, ts(0, 4096)])
    wv_half(0)
    nc.sync.dma_start(ht16[:, ts(1, 4096)], hbt16[:, ts(1, 4096)])
    nc.sync.dma_start(ht8[:, ts(1, 4096)], hbt8[:, ts(1, 4096)])
    nc.sync.dma_start(ht16[:, ts(2, 4096)], hbt16[:, ts(2, 4096)])
    nc.sync.dma_start(ht8[:, ts(2, 4096)], hbt8[:, ts(2, 4096)])
    nc.sync.dma_start(ht16[:, ts(3, 4096)], hbt16[:, ts(3, 4096)])
    nc.sync.dma_start(ht8[:, ts(3, 4096)], hbt8[:, ts(3, 4096)])
    wv_half(1)
    nc.sync.dma_start(wo_sb[:], wo8[:, :])
    hbres = [rpool.tile([128, D_MODEL], BF16, tag=f"hbres{i % 2}",
                        name=f"hbres{i}") for i in range(ISUB)]
    for i in range(ISUB):
        nc.sync.dma_start(hbres[i][:], hb[ts(i, 128), :])

    # ---- emission helpers ----
    def kq_tile(p, jb, is_q):
        """One [128, 512] projection tile of K^T or Q^T (bf16 matmuls).
        Output partitions = (2 heads of pair p) x 64 dims."""
        w, dst, blk = (wq_sb, qt, LOCAL) if is_q else (wk_sb, kt, SEQ)
        ps = psum.tile([128, 512], F32, tag="proj", name="kqps")
        for c in range(CC):
            nc.tensor.matmul(
                ps[:], w[:, c * 1024 + p * 128: c * 1024 + (p + 1) * 128],
                ht16[:, jb * 4096 + c * 512: jb * 4096 + (c + 1) * 512],
                start=(c == 0), stop=(c == CC - 1),
            )
        nc.vector.tensor_copy(dst[:, p * blk + jb * 512:
                                  p * blk + jb * 512 + 512], ps[:])

    def v_tile(j16, half):
        """V projection for one 128-token j-chunk, heads 8h..8h+7 (fp8
        DR)."""
        jb, t0 = divmod(j16, 4)
        ps = psum.tile([128, 512], F32, tag="proj", name="vps")
        for cp in range(CC // 2):
            lhs = _slab3(ht8[:], jb * 4096 + (2 * cp) * 512 + t0 * 128,
                         512, 128)
            rhs = _slab3(wv_sb[:], (2 * cp) * 1024 + half * 512,
                         1024, 512)
            nc.tensor.matmul(ps[:], lhs, rhs, start=(cp == 0),
                             stop=(cp == CC // 2 - 1), perf_mode=DR)
        # strided scatter into v8: head n = 8*half+k gets cols
        # n*1280 + j16*80 .. +64
        base = (8 * half) * (J16 * 80) + j16 * 80
        a = v8[:, base:base + 1]
        dst = bass.AP(tensor=a.tensor, offset=a.offset,
                      ap=[list(a.ap[0]), [J16 * 80, 8], [1, 64]])
        nc.vector.tensor_copy(dst, ps[:])

    acc_of = {}

    def unit(h, ib, u):
        """Scores + exp + PV for head h, i-block ib, jc-pair u."""
        p, hh = divmod(h, 2)
        s4 = psum.tile([128, 1024], F32, tag="s4", name="s4")
        for uu in range(2):
            jc = 2 * u + uu
            lhs = _slab3(kt[ts(hh, 64), :], p * SEQ + jc * 128, 0, 128)
            rhs = _slab3(qt[ts(hh, 64), :], p * LOCAL + ib * 512, 0, 512)
            nc.tensor.matmul(s4[:, ts(uu, 512)], lhs, rhs,
                             start=True, stop=True, perf_mode=DR)
        e = epool.tile([128, 1024], FP8, tag="e", name="e")
        nc.scalar.activation(e[:], s4[:], AF.Exp, bias=lnb_t[:])
        acc = acc_of[(h, ib)]
        lhs = _slab3(v8[:], h * (J16 * 80) + (2 * u) * 80, 80, 65)
        rhs = _slab3(e[:], 0, 512, 512)
        nc.tensor.matmul(acc[0:65, :], lhs, rhs, start=(u == 0),
                         stop=(u == NU - 1), perf_mode=DR)

    def normalize(h, ib):
        acc = acc_of.pop((h, ib))
        rec = spool.tile([1, 512], F32, tag="rec", name="rec")
        nc.vector.reciprocal(rec[:], acc[64:65, :])
        rb = spool.tile([64, 512], F32, tag="recb", name="rb")
        nc.gpsimd.partition_broadcast(rb[:], rec[:])
        p, hh = divmod(h, 2)
        nc.vector.tensor_mul(at_ib[ib][ts(hh, 64), ts(p, 512)],
                             acc[0:64, :], rb[:])

    def pair(p, pre_map=None, post_map=None):
        """Attention for heads 2p, 2p+1 with their units interleaved (the
        two exps per u-slot double the pipeline window for carried work).
        pre_map[(ib, u)] = emitters run BEFORE that u-slot; post_map
        likewise after the slot."""
        pre_map = pre_map or {}
        post_map = post_map or {}
        h0, h1 = 2 * p, 2 * p + 1
        for ib in range(IBL):
            acc_of[(h0, ib)] = psum.tile([128, 512], F32, tag="acc",
                                         name="acc")
            acc_of[(h1, ib)] = psum.tile([128, 512], F32, tag="acc",
                                         name="acc")
            for u in range(NU):
                for fn in pre_map.get((ib, u), ()):
                    fn()
                unit(h0, ib, u)
                unit(h1, ib, u)
                for fn in post_map.get((ib, u), ()):
                    fn()
            normalize(h0, ib)
            normalize(h1, ib)

    # Output blocks in three phases so the per-block Ln/Exp rstd pairs
    # don't thrash the ACT function table against the attention exps:
    # A) projection + residual + bn stats (no ACT), B) one batched
    # Ln/Exp over all 8 variances after the last exp, C) normalize+store.
    x_of = [None] * ISUB
    mv_of = [None] * ISUB
    vbat = consts.tile([128, ISUB], F32, name="vbat")
    rbat = consts.tile([128, ISUB], F32, name="rbat")

    def wo_a(isub):
        ib, t = divmod(isub, 4)
        x = xpool.tile([128, D_MODEL], F32, tag=f"x{isub}", name="x")
        x_of[isub] = x
        for dm in range(2):
            ops = psum.tile([128, 512], F32, tag="proj", name="ops")
            for qp in range(4):
                lhs = _slab3(at_ib[ib][:], (2 * qp) * 512 + t * 128,
                             512, 128)
                rhs = _slab3(wo_sb[:], (2 * qp) * 1024 + dm * 512,
                             1024, 512)
                nc.tensor.matmul(ops[:], lhs, rhs, start=(qp == 0),
                                 stop=(qp == 3), perf_mode=DR)
            nc.vector.tensor_add(x[:, ts(dm, 512)], ops[:],
                                 hbres[isub][:, ts(dm, 512)])
        stats = spool.tile([128, 2, 6], F32, tag="bnst", name="st")
        mv = spool.tile([128, 2], F32, tag=f"bnmv{isub}", name="mv")
        mv_of[isub] = mv
        for gg in range(2):
            nc.vector.bn_stats(stats[:, gg, :], x[:, ts(gg, 512)])
        nc.vector.bn_aggr(mv[:], stats[:])
        nc.vector.tensor_copy(vbat[:, isub:isub + 1], mv[:, 1:2])

    def wo_b():
        nc.scalar.activation(rbat[:], vbat[:], AF.Sqrt, bias=eps_t[:])
        nc.vector.reciprocal(rbat[:], rbat[:])

    def wo_c(isub):
        x = x_of[isub]
        nc.vector.tensor_scalar(
            x[:], x[:], mv_of[isub][:, 0:1], rbat[:, isub:isub + 1],
            op0=mybir.AluOpType.subtract, op1=mybir.AluOpType.mult)
        if affine:
            nc.vector.tensor_mul(x[:], x[:], gamma_b[:])
            nc.vector.tensor_add(x[:], x[:], beta_b[:])
        nc.sync.dma_start(out[ts(isub, 128), :], x[:])

    # ---- schedule ----
    def V(j16, half=0):
        return lambda: v_tile(j16, half)

    def K(p, jb):
        return lambda: kq_tile(p, jb, False)

    def Q(p, ib):
        return lambda: kq_tile(p, ib, True)

    # Lead-in: minimum work before the first exp can fire.
    kq_tile(0, 0, False)
    kq_tile(0, 0, True)

    # Each pair self-carries its own later K j-blocks (needed at u-slot
    # 2b) and Q i-block 1, plus the NEXT pair's first K/Q; pair 0 also
    # carries all 16 V chunks (V(2u), V(2u+1) before u-slot u for PV).
    p0_pre = {
        (0, 0): [V(0), V(1)],
        (0, 1): [V(2), V(3)],
        (0, 2): [V(4), V(5), K(0, 1)],
        (0, 3): [V(6), V(7)],
        (0, 4): [V(8), V(9), K(0, 2)],
        (0, 5): [V(10), V(11)],
        (0, 6): [V(12), V(13), K(0, 3)],
        (0, 7): [V(14), V(15), Q(0, 1)],
        (1, 1): [K(1, 0)],
        (1, 4): [Q(1, 0)],
    }
    pair(0, p0_pre)

    # Pairs 1..7: self-carry K jb 1-3 at u-slots 2,4,6 and Q ib1 at
    # slot 7; hand the next pair its first K/Q during ib 1.  The last
    # pair interleaves the first output-projection blocks into its ib-1
    # phase (their at-deps complete at ib-0's end).
    for p in range(1, N_PAIR):
        pre_map = {
            (0, 2): [K(p, 1)],
            (0, 4): [K(p, 2)],
            (0, 6): [K(p, 3)],
            (0, 7): [Q(p, 1)],
        }
        post_map = {}
        if p in (1, 2):
            # heads 8-15's V chunks, needed from pair 4 on
            for u in range(NU):
                pre_map.setdefault((1, u), []).append(
                    V(8 * (p - 1) + u, 1))
        if p < N_PAIR - 1:
            pre_map.setdefault((1, 1), []).append(K(p + 1, 0))
            pre_map.setdefault((1, 4), []).append(Q(p + 1, 0))
        else:
            for isub in range(4):
                post_map.setdefault((1, 2 * isub), []).append(
                    lambda isub=isub: wo_a(isub))
        pair(p, pre_map, post_map)
    for isub in range(4, ISUB):
        wo_a(isub)
    wo_b()
    for isub in range(ISUB):
        wo_c(isub)


_program_cache = {}


def _get_program(affine=False):
    key = ("nc", affine)
    if key not in _program_cache:
        _program_cache[key] = build_program(affine)
    return _program_cache[key]


def _chunk_cols(w):
    """[1024, 1024] -> [128, 8*1024] with col c*1024+m = w[128c+p, m]."""
    return np.ascontiguousarray(
        w.reshape(CC, 128, D_MODEL).transpose(1, 0, 2).reshape(128, -1))


def _h_layout(hp, dt):
    """h_perm [2048, 1024] -> [128, jb(4) x c(8) x 512] in dtype dt."""
    a = hp.astype(dt)
    # [jb, t', c, p] -> [p, jb, c, t']
    a = a.reshape(JB, 512, CC, 128).transpose(3, 0, 2, 1)
    return np.ascontiguousarray(a.reshape(128, -1))


def _shard_inputs(h, Wq, Wkv, Wo, gamma, beta):
    h = np.asarray(h, np.float32)
    Wq = np.asarray(Wq, np.float32)
    Wkv = np.asarray(Wkv, np.float32)
    Wo = np.asarray(Wo, np.float32)
    gamma = np.asarray(gamma, np.float32)
    beta = np.asarray(beta, np.float32)

    # scores DoubleRow contracts the same slab twice -> fold an extra
    # 1/2 into the Wq scale
    scale = 0.5 / np.sqrt(D_HEAD)
    Wk = Wkv[:, :N_HEAD * D_HEAD]
    Wv = Wkv[:, N_HEAD * D_HEAD:]
    wq16 = _chunk_cols(Wq * scale).astype(ml_dtypes.bfloat16)
    wk16 = _chunk_cols(Wk).astype(ml_dtypes.bfloat16)
    wv8 = _chunk_cols(Wv).astype(ml_dtypes.float8_e4m3)
    wo8 = _chunk_cols(Wo).astype(ml_dtypes.float8_e4m3)

    in_maps = []
    for core in range(N_CORES):
        b, r = divmod(core, 2)
        hb_full = h[:, b, :]
        if r == 0:
            hp = hb_full
        else:
            hp = np.concatenate([hb_full[LOCAL:], hb_full[:LOCAL]], axis=0)
        in_maps.append({
            "hbt16": _h_layout(hp, ml_dtypes.bfloat16),
            "hbt8": _h_layout(hp, ml_dtypes.float8_e4m3),
            "wq16": wq16, "wk16": wk16, "wv8": wv8, "wo8": wo8,
            "hb": np.ascontiguousarray(hp[:LOCAL].astype(ml_dtypes.bfloat16)),
            "gamma": gamma, "beta": beta,
        })
    return in_maps


def kernel(h, Wq, Wkv, Wo, gamma, beta, _trace=False):
    gamma = np.asarray(gamma, np.float32)
    beta = np.asarray(beta, np.float32)
    affine = not (np.all(gamma == 1.0) and np.all(beta == 0.0))
    nc = _get_program(affine)
    in_maps = _shard_inputs(h, Wq, Wkv, Wo, gamma, beta)
    res = run_bass_kernel_spmd(nc, in_maps, list(range(N_CORES)),
                               trace=_trace)
    if _trace:
        kernel.last_results = res

    out = np.empty((SEQ, BATCH, D_MODEL), np.float32)
    for core in range(N_CORES):
        b, r = divmod(core, 2)
        out[r * LOCAL:(r + 1) * LOCAL, b, :] = res.results[core]["out"]
    return out


# revision 17
# speedup vs baseline: 1.4478x; 1.0103x over previous
"""Trainium2 Bass kernel for nn_MultiHeadAttn_80126909874682.

Full MHA layer: QKV projection -> 16-head attention (seq 2048) -> output
projection -> residual -> LayerNorm, over h [2048, 4, 1024] fp32.

Sharding (8 NeuronCores, zero collectives):
  core c -> batch b = c // 2, token-half r = c % 2.
  Each core computes K/V for all 2048 tokens of its batch (all 16 heads)
  and Q / attention / output projection / LayerNorm for its 1024 local
  tokens only.  The per-core inputs are permuted so the core's local
  tokens come first; attention is invariant to the j-permutation of K/V.

v3 (fp8 DoubleRow): the attention-side matmuls (V projection, QK^T
scores, PV, output projection) run in fp8e4 with the DoubleRow perf
mode (two k-tile slabs contracted per instruction at 0.5 cycles/row).
Q/K projections stay bf16 (fp8 there dominates the output error).  The
scores matmul has only a 64-deep contraction, so its two DoubleRow
slabs alias the same data via stride-0 APs with the 2x folded into the
host-side Wq scale.  Exp runs on the ACT engine with a -ln(16) bias
(keeps e below the fp8e4 max); a ones-column appended to V makes the PV
matmul also emit the softmax denominators.  PSUM: 2x[128,1024] score
tiles + 2x[128,512] PV accumulators + 2x[128,512] projection tiles = 8
banks exactly.  ACT (256 exps) is the critical path; everything else
(copies, normalize, LayerNorm) lives on DVE/Pool/SP.
"""

import os
import sys

os.environ.setdefault("JAX_PLATFORMS", "axon")
sys.path.insert(0, "/opt/trn_rl_repo")

import numpy as np
import ml_dtypes

import concourse.bass as bass
import concourse.tile as tile
from concourse import bacc, mybir
from concourse.bass import ts
from concourse.bass_utils import run_bass_kernel_spmd

N_HEAD = 16
D_MODEL = 1024
D_HEAD = 64
SEQ = 2048
BATCH = 4
EPS = 1e-5
N_CORES = 8

LOCAL = SEQ // 2            # tokens owned per core (1024)
CC = D_MODEL // 128         # dmodel contraction chunks (8)
N_PAIR = N_HEAD // 2        # head pairs (8)
JB = SEQ // 512             # 512-token j blocks (4)
J16 = SEQ // 128            # 128-token j chunks (16)
NU = J16 // 2               # jc pairs per (head, iblock) unit (8)
IBL = LOCAL // 512          # local 512-token i blocks (2)
ISUB = LOCAL // 128         # local 128-token i sub tiles (8)
LN16 = float(np.log(16.0))

F32 = mybir.dt.float32
BF16 = mybir.dt.bfloat16
FP8 = mybir.dt.float8e4
AF = mybir.ActivationFunctionType
DR = mybir.MatmulPerfMode.DoubleRow


def _slab3(ap2, col0, slab_stride, n):
    """[P, n] view at col0 with an extra middle slab dim [slab_stride, 2]."""
    a = ap2[:, col0:col0 + 1]
    return bass.AP(
        tensor=a.tensor, offset=a.offset,
        ap=[list(a.ap[0]), [slab_stride, 2], [1, n]],
    )


def build_program(affine):
    nc = bacc.Bacc()

    hbt16 = nc.declare_dram_parameter("hbt16", [128, JB * CC * 512], BF16,
                                      isOutput=False)
    hbt8 = nc.declare_dram_parameter("hbt8", [128, JB * CC * 512], FP8,
                                     isOutput=False)
    wq16 = nc.declare_dram_parameter("wq16", [128, CC * D_MODEL], BF16,
                                     isOutput=False)
    wk16 = nc.declare_dram_parameter("wk16", [128, CC * D_MODEL], BF16,
                                     isOutput=False)
    wv8 = nc.declare_dram_parameter("wv8", [128, CC * D_MODEL], FP8,
                                    isOutput=False)
    wo8 = nc.declare_dram_parameter("wo8", [128, CC * D_MODEL], FP8,
                                    isOutput=False)
    hb = nc.declare_dram_parameter("hb", [LOCAL, D_MODEL], BF16,
                                   isOutput=False)
    gamma = nc.declare_dram_parameter("gamma", [D_MODEL], F32, isOutput=False)
    beta = nc.declare_dram_parameter("beta", [D_MODEL], F32, isOutput=False)
    out = nc.declare_dram_parameter("out", [LOCAL, D_MODEL], F32,
                                    isOutput=True)

    with tile.TileContext(nc) as tc:
        with (
            tc.tile_pool(name="consts", bufs=1) as consts,
            tc.tile_pool(name="weights", bufs=1) as wpool,
            tc.tile_pool(name="hbt", bufs=1) as hpool,
            tc.tile_pool(name="ktq", bufs=1) as kpool,
            tc.tile_pool(name="vsb", bufs=1) as vpool,
            tc.tile_pool(name="attn", bufs=1) as apool,
            tc.tile_pool(name="exp", bufs=4) as epool,
            tc.tile_pool(name="small", bufs=3) as spool,
            tc.tile_pool(name="xstage", bufs=1) as xpool,
            tc.tile_pool(name="hbres", bufs=2) as rpool,
            tc.tile_pool(name="psum", bufs=2, space="PSUM") as psum,
        ):
            _emit(nc, hbt16, hbt8, wq16, wk16, wv8, wo8, hb, gamma, beta,
                  out, consts, wpool, hpool, kpool, vpool, apool, epool,
                  spool, xpool, rpool, psum, affine)

    nc.finalize()
    return nc


def _emit(nc, hbt16, hbt8, wq16, wk16, wv8, wo8, hb, gamma, beta, out,
          consts, wpool, hpool, kpool, vpool, apool, epool, spool, xpool,
          rpool, psum, affine):
    # ---- SBUF tiles ----
    eps_t = consts.tile([128, 1], F32)
    nc.vector.memset(eps_t[:], EPS)
    lnb_t = consts.tile([128, 1], F32)
    nc.vector.memset(lnb_t[:], -LN16)
    if affine:
        gamma_b = consts.tile([128, D_MODEL], F32)
        beta_b = consts.tile([128, D_MODEL], F32)
        g_ap, b_ap = gamma.ap(), beta.ap()
        nc.gpsimd.dma_start(
            out=gamma_b[:],
            in_=bass.AP(tensor=g_ap.tensor, offset=g_ap.offset,
                        ap=[[0, 128], [1, D_MODEL]]))
        nc.gpsimd.dma_start(
            out=beta_b[:],
            in_=bass.AP(tensor=b_ap.tensor, offset=b_ap.offset,
                        ap=[[0, 128], [1, D_MODEL]]))

    wk_sb = wpool.tile([128, CC * D_MODEL], BF16, name="wk")
    wq_sb = wpool.tile([128, CC * D_MODEL], BF16, name="wq")
    wv_sb = wpool.tile([128, CC * D_MODEL], FP8, name="wv")
    wo_sb = wpool.tile([128, CC * D_MODEL], FP8, name="wo")
    ht16 = hpool.tile([128, JB * CC * 512], BF16, name="ht16")
    ht8 = hpool.tile([128, JB * CC * 512], FP8, name="ht8")
    # kt: per pair [128p = 2 heads x 64 d] x [pair(8) x j(2048)] fp8
    kt = kpool.tile([128, N_PAIR * SEQ], FP8, name="kt")
    qt = kpool.tile([128, N_PAIR * LOCAL], FP8, name="qt")
    # v8: per head 16 j-chunks x (64 dims + ones col + 15 pad)
    # (DoubleRow slab strides must be multiples of 16 elements)
    v8 = vpool.tile([128, N_HEAD * J16 * 80], FP8, name="v8")
    # at: per i-block [128p = pair dims] x [pair(8) x i(512)] fp8
    # (two tiles so the output projection's reads of i-block 0 don't
    # serialize behind i-block-1 normalize writes via tile-granular deps)
    at_ib = [apool.tile([128, 8 * 512], FP8, name=f"at{ib}")
             for ib in range(IBL)]

    # ones columns of v8 (col h*1280 + j*80 + 64)
    for n in range(N_HEAD):
        base = n * (J16 * 80) + 64
        a = v8[:, base:base + 1]
        nc.gpsimd.memset(
            bass.AP(tensor=a.tensor, offset=a.offset,
                    ap=[list(a.ap[0]), [80, J16], [1, 1]]), 1.0)

    # ---- DMA loads ----
    # Single HWDGE (sync) queue, priority order: the DMA engines serialize
    # transfers, so the lead-in critical path (ht16 jb0 -> wk -> wq) goes
    # first; per-chunk weight loads let the projection matmuls accumulate
    # as chunks land.
    def wv_half(half):
        a = wv_sb[:, half * 512:half * 512 + 1]
        dst = bass.AP(tensor=a.tensor, offset=a.offset,
                      ap=[list(a.ap[0]), [1024, CC], [1, 512]])
        s = wv8.ap()
        srcap = bass.AP(tensor=s.tensor, offset=s.offset + half * 512,
                        ap=[list(s.ap[0]), [1024, CC], [1, 512]])
        nc.sync.dma_start(dst, srcap)

    nc.sync.dma_start(ht16[:, ts(0, 4096)], hbt16[:, ts(0, 4096)])
    for c in range(CC):
        nc.sync.dma_start(wk_sb[:, ts(c, 1024)], wk16[:, ts(c, 1024)])
    for c in range(CC):
        nc.sync.dma_start(wq_sb[:, ts(c, 1024)], wq16[:, ts(c, 1024)])
    nc.sync.dma_start(ht8[:, ts(0, 4096)], hbt8[:, ts(0, 4096)])
    wv_half(0)
    nc.sync.dma_start(ht16[:, ts(1, 4096)], hbt16[:, ts(1, 4096)])
    nc.sync.dma_start(ht8[:, ts(1, 4096)], hbt8[:, ts(1, 4096)])
    nc.sync.dma_start(ht16[:, ts(2, 4096)], hbt16[:, ts(2, 4096)])
    nc.sync.dma_start(ht8[:, ts(2, 4096)], hbt8[:, ts(2, 4096)])
    nc.sync.dma_start(ht16[:, ts(3, 4096)], hbt16[:, ts(3, 4096)])
    nc.sync.dma_start(ht8[:, ts(3, 4096)], hbt8[:, ts(3, 4096)])
    wv_half(1)
    nc.sync.dma_start(wo_sb[:], wo8[:, :])
    hbres = [rpool.tile([128, D_MODEL], BF16, tag=f"hbres{i % 2}",
                        name=f"hbres{i}") for i in range(ISUB)]
    for i in range(ISUB):
        nc.sync.dma_start(hbres[i][:], hb[ts(i, 128), :])

    # ---- emission helpers ----
    def kq_tile(p, jb, is_q):
        """One [128, 512] projection tile of K^T or Q^T (bf16 matmuls).
        Output partitions = (2 heads of pair p) x 64 dims."""
        w, dst, blk = (wq_sb, qt, LOCAL) if is_q else (wk_sb, kt, SEQ)
        ps = psum.tile([128, 512], F32, tag="proj", name="kqps")
        for c in range(CC):
            nc.tensor.matmul(
                ps[:], w[:, c * 1024 + p * 128: c * 1024 + (p + 1) * 128],
                ht16[:, jb * 4096 + c * 512: jb * 4096 + (c + 1) * 512],
                start=(c == 0), stop=(c == CC - 1),
            )
        nc.vector.tensor_copy(dst[:, p * blk + jb * 512:
                                  p * blk + jb * 512 + 512], ps[:])

    def v_tile(j16, half):
        """V projection for one 128-token j-chunk, heads 8h..8h+7 (fp8
        DR)."""
        jb, t0 = divmod(j16, 4)
        ps = psum.tile([128, 512], F32, tag="proj", name="vps")
        for cp in range(CC // 2):
            lhs = _slab3(ht8[:], jb * 4096 + (2 * cp) * 512 + t0 * 128,
                         512, 128)
            rhs = _slab3(wv_sb[:], (2 * cp) * 1024 + half * 512,
                         1024, 512)
            nc.tensor.matmul(ps[:], lhs, rhs, start=(cp == 0),
                             stop=(cp == CC // 2 - 1), perf_mode=DR)
        # strided scatter into v8: head n = 8*half+k gets cols
        # n*1280 + j16*80 .. +64
        base = (8 * half) * (J16 * 80) + j16 * 80
        a = v8[:, base:base + 1]
        dst = bass.AP(tensor=a.tensor, offset=a.offset,
                      ap=[list(a.ap[0]), [J16 * 80, 8], [1, 64]])
        nc.vector.tensor_copy(dst, ps[:])

    acc_of = {}

    def unit(h, ib, u):
        """Scores + exp + PV for head h, i-block ib, jc-pair u."""
        p, hh = divmod(h, 2)
        s4 = psum.tile([128, 1024], F32, tag="s4", name="s4")
        for uu in range(2):
            jc = 2 * u + uu
            lhs = _slab3(kt[ts(hh, 64), :], p * SEQ + jc * 128, 0, 128)
            rhs = _slab3(qt[ts(hh, 64), :], p * LOCAL + ib * 512, 0, 512)
            nc.tensor.matmul(s4[:, ts(uu, 512)], lhs, rhs,
                             start=True, stop=True, perf_mode=DR)
        e = epool.tile([128, 1024], FP8, tag="e", name="e")
        nc.scalar.activation(e[:], s4[:], AF.Exp, bias=lnb_t[:])
        acc = acc_of[(h, ib)]
        lhs = _slab3(v8[:], h * (J16 * 80) + (2 * u) * 80, 80, 65)
        rhs = _slab3(e[:], 0, 512, 512)
        nc.tensor.matmul(acc[0:65, :], lhs, rhs, start=(u == 0),
                         stop=(u == NU - 1), perf_mode=DR)

    def normalize(h, ib):
        acc = acc_of.pop((h, ib))
        rec = spool.tile([1, 512], F32, tag="rec", name="rec")
        nc.vector.reciprocal(rec[:], acc[64:65, :])
        rb = spool.tile([64, 512], F32, tag="recb", name="rb")
        nc.gpsimd.partition_broadcast(rb[:], rec[:])
        p, hh = divmod(h, 2)
        nc.vector.tensor_mul(at_ib[ib][ts(hh, 64), ts(p, 512)],
                             acc[0:64, :], rb[:])

    def pair(p, pre_map=None, post_map=None):
        """Attention for heads 2p, 2p+1 with their units interleaved (the
        two exps per u-slot double the pipeline window for carried work).
        pre_map[(ib, u)] = emitters run BEFORE that u-slot; post_map
        likewise after the slot."""
        pre_map = pre_map or {}
        post_map = post_map or {}
        h0, h1 = 2 * p, 2 * p + 1
        for ib in range(IBL):
            acc_of[(h0, ib)] = psum.tile([128, 512], F32, tag="acc",
                                         name="acc")
            acc_of[(h1, ib)] = psum.tile([128, 512], F32, tag="acc",
                                         name="acc")
            for u in range(NU):
                for fn in pre_map.get((ib, u), ()):
                    fn()
                unit(h0, ib, u)
                unit(h1, ib, u)
                for fn in post_map.get((ib, u), ()):
                    fn()
            normalize(h0, ib)
            normalize(h1, ib)

    # Output blocks in three phases so the per-block Ln/Exp rstd pairs
    # don't thrash the ACT function table against the attention exps:
    # A) projection + residual + bn stats (no ACT), B) one batched
    # Ln/Exp over all 8 variances after the last exp, C) normalize+store.
    x_of = [None] * ISUB
    mv_of = [None] * ISUB
    vbat = consts.tile([128, ISUB], F32, name="vbat")
    rbat = consts.tile([128, ISUB], F32, name="rbat")

    def wo_a(isub):
        ib, t = divmod(isub, 4)
        x = xpool.tile([128, D_MODEL], F32, tag=f"x{isub}", name="x")
        x_of[isub] = x
        for dm in range(2):
            ops = psum.tile([128, 512], F32, tag="proj", name="ops")
            for qp in range(4):
                lhs = _slab3(at_ib[ib][:], (2 * qp) * 512 + t * 128,
                             512, 128)
                rhs = _slab3(wo_sb[:], (2 * qp) * 1024 + dm * 512,
                             1024, 512)
                nc.tensor.matmul(ops[:], lhs, rhs, start=(qp == 0),
                                 stop=(qp == 3), perf_mode=DR)
            nc.vector.tensor_add(x[:, ts(dm, 512)], ops[:],
                                 hbres[isub][:, ts(dm, 512)])
        stats = spool.tile([128, 2, 6], F32, tag="bnst", name="st")
        mv = spool.tile([128, 2], F32, tag=f"bnmv{isub}", name="mv")
        mv_of[isub] = mv
        for gg in range(2):
            nc.vector.bn_stats(stats[:, gg, :], x[:, ts(gg, 512)])
        nc.vector.bn_aggr(mv[:], stats[:])
        nc.vector.tensor_copy(vbat[:, isub:isub + 1], mv[:, 1:2])

    def wo_b():
        nc.scalar.activation(rbat[:], vbat[:], AF.Sqrt, bias=eps_t[:])
        nc.vector.reciprocal(rbat[:], rbat[:])

    def wo_c(isub):
        x = x_of[isub]
        nc.vector.tensor_scalar(
            x[:], x[:], mv_of[isub][:, 0:1], rbat[:, isub:isub + 1],
            op0=mybir.AluOpType.subtract, op1=mybir.AluOpType.mult)
        if affine:
            nc.vector.tensor_mul(x[:], x[:], gamma_b[:])
            nc.vector.tensor_add(x[:], x[:], beta_b[:])
        nc.sync.dma_start(out[ts(isub, 128), :], x[:])

    # ---- schedule ----
    def V(j16, half=0):
        return lambda: v_tile(j16, half)

    def K(p, jb):
        return lambda: kq_tile(p, jb, False)

    def Q(p, ib):
        return lambda: kq_tile(p, ib, True)

    # Lead-in: minimum work before the first exp can fire.
    kq_tile(0, 0, False)
    kq_tile(0, 0, True)

    # Each pair self-carries its own later K j-blocks (needed at u-slot
    # 2b) and Q i-block 1, plus the NEXT pair's first K/Q; pair 0 also
    # carries all 16 V chunks (V(2u), V(2u+1) before u-slot u for PV).
    p0_pre = {
        (0, 0): [V(0), V(1)],
        (0, 1): [V(2), V(3)],
        (0, 2): [K(0, 1), V(4), V(5)],
        (0, 3): [V(6), V(7)],
        (0, 4): [K(0, 2), V(8), V(9)],
        (0, 5): [V(10), V(11)],
        (0, 6): [K(0, 3), V(12), V(13)],
        (0, 7): [Q(0, 1), V(14), V(15)],
        (1, 1): [K(1, 0)],
        (1, 4): [Q(1, 0)],
    }
    pair(0, p0_pre)

    # Pairs 1..7: self-carry K jb 1-3 at u-slots 2,4,6 and Q ib1 at
    # slot 7; hand the next pair its first K/Q during ib 1.  The last
    # pair interleaves the first output-projection blocks into its ib-1
    # phase (their at-deps complete at ib-0's end).
    for p in range(1, N_PAIR):
        pre_map = {
            (0, 2): [K(p, 1)],
            (0, 4): [K(p, 2)],
            (0, 6): [K(p, 3)],
            (0, 7): [Q(p, 1)],
        }
        post_map = {}
        if p in (1, 2):
            # heads 8-15's V chunks, needed from pair 4 on
            for u in range(NU):
                pre_map.setdefault((1, u), []).append(
                    V(8 * (p - 1) + u, 1))
        if p < N_PAIR - 1:
            pre_map.setdefault((1, 1), []).append(K(p + 1, 0))
            pre_map.setdefault((1, 4), []).append(Q(p + 1, 0))
        else:
            for isub in range(4):
                post_map.setdefault((1, isub), []).append(
                    lambda isub=isub: wo_a(isub))
        pair(p, pre_map, post_map)
    for isub in range(4, ISUB):
        wo_a(isub)
    wo_b()
    for isub in range(ISUB):
        wo_c(isub)


_program_cache = {}


def _get_program(affine=False):
    key = ("nc", affine)
    if key not in _program_cache:
        _program_cache[key] = build_program(affine)
    return _program_cache[key]


def _chunk_cols(w):
    """[1024, 1024] -> [128, 8*1024] with col c*1024+m = w[128c+p, m]."""
    return np.ascontiguousarray(
        w.reshape(CC, 128, D_MODEL).transpose(1, 0, 2).reshape(128, -1))


def _h_layout(hp, dt):
    """h_perm [2048, 1024] -> [128, jb(4) x c(8) x 512] in dtype dt."""
    a = hp.astype(dt)
    # [jb, t', c, p] -> [p, jb, c, t']
    a = a.reshape(JB, 512, CC, 128).transpose(3, 0, 2, 1)
    return np.ascontiguousarray(a.reshape(128, -1))


def _shard_inputs(h, Wq, Wkv, Wo, gamma, beta):
    h = np.asarray(h, np.float32)
    Wq = np.asarray(Wq, np.float32)
    Wkv = np.asarray(Wkv, np.float32)
    Wo = np.asarray(Wo, np.float32)
    gamma = np.asarray(gamma, np.float32)
    beta = np.asarray(beta, np.float32)

    # scores DoubleRow contracts the same slab twice -> fold an extra
    # 1/2 into the Wq scale
    scale = 0.5 / np.sqrt(D_HEAD)
    Wk = Wkv[:, :N_HEAD * D_HEAD]
    Wv = Wkv[:, N_HEAD * D_HEAD:]
    wq16 = _chunk_cols(Wq * scale).astype(ml_dtypes.bfloat16)
    wk16 = _chunk_cols(Wk).astype(ml_dtypes.bfloat16)
    wv8 = _chunk_cols(Wv).astype(ml_dtypes.float8_e4m3)
    wo8 = _chunk_cols(Wo).astype(ml_dtypes.float8_e4m3)

    in_maps = []
    for core in range(N_CORES):
        b, r = divmod(core, 2)
        hb_full = h[:, b, :]
        if r == 0:
            hp = hb_full
        else:
            hp = np.concatenate([hb_full[LOCAL:], hb_full[:LOCAL]], axis=0)
        in_maps.append({
            "hbt16": _h_layout(hp, ml_dtypes.bfloat16),
            "hbt8": _h_layout(hp, ml_dtypes.float8_e4m3),
            "wq16": wq16, "wk16": wk16, "wv8": wv8, "wo8": wo8,
            "hb": np.ascontiguousarray(hp[:LOCAL].astype(ml_dtypes.bfloat16)),
            "gamma": gamma, "beta": beta,
        })
    return in_maps


def kernel(h, Wq, Wkv, Wo, gamma, beta, _trace=False):
    gamma = np.asarray(gamma, np.float32)
    beta = np.asarray(beta, np.float32)
    affine = not (np.all(gamma == 1.0) and np.all(beta == 0.0))
    nc = _get_program(affine)
    in_maps = _shard_inputs(h, Wq, Wkv, Wo, gamma, beta)
    res = run_bass_kernel_spmd(nc, in_maps, list(range(N_CORES)),
                               trace=_trace)
    if _trace:
        kernel.last_results = res

    out = np.empty((SEQ, BATCH, D_MODEL), np.float32)
    for core in range(N_CORES):
        b, r = divmod(core, 2)
        out[r * LOCAL:(r + 1) * LOCAL, b, :] = res.results[core]["out"]
    return out


# revision 20
# speedup vs baseline: 1.5244x; 1.0529x over previous
"""Trainium2 Bass kernel for nn_MultiHeadAttn_80126909874682.

Full MHA layer: QKV projection -> 16-head attention (seq 2048) -> output
projection -> residual -> LayerNorm, over h [2048, 4, 1024] fp32.

Sharding (8 NeuronCores, zero collectives):
  core c -> batch b = c // 2, token-half r = c % 2.
  Each core computes K/V for all 2048 tokens of its batch (all 16 heads)
  and Q / attention / output projection / LayerNorm for its 1024 local
  tokens only.  The per-core inputs are permuted so the core's local
  tokens come first; attention is invariant to the j-permutation of K/V.

v3 (fp8 DoubleRow): the attention-side matmuls (V projection, QK^T
scores, PV, output projection) run in fp8e4 with the DoubleRow perf
mode (two k-tile slabs contracted per instruction at 0.5 cycles/row).
Q/K projections stay bf16 (fp8 there dominates the output error).  The
scores matmul has only a 64-deep contraction, so its two DoubleRow
slabs alias the same data via stride-0 APs with the 2x folded into the
host-side Wq scale.  Exp runs on the ACT engine with a -ln(16) bias
(keeps e below the fp8e4 max); a ones-column appended to V makes the PV
matmul also emit the softmax denominators.  PSUM: 2x[128,1024] score
tiles + 2x[128,512] PV accumulators + 2x[128,512] projection tiles = 8
banks exactly.  ACT (256 exps) is the critical path; everything else
(copies, normalize, LayerNorm) lives on DVE/Pool/SP.
"""

import os
import sys

os.environ.setdefault("JAX_PLATFORMS", "axon")
sys.path.insert(0, "/opt/trn_rl_repo")

import numpy as np
import ml_dtypes

import concourse.bass as bass
import concourse.tile as tile
from concourse import bacc, mybir
from concourse.bass import ts
from concourse.bass_utils import run_bass_kernel_spmd

N_HEAD = 16
D_MODEL = 1024
D_HEAD = 64
SEQ = 2048
BATCH = 4
EPS = 1e-5
N_CORES = 8

LOCAL = SEQ // 2            # tokens owned per core (1024)
CC = D_MODEL // 128         # dmodel contraction chunks (8)
N_PAIR = N_HEAD // 2        # head pairs (8)
JB = SEQ // 512             # 512-token j blocks (4)
J16 = SEQ // 128            # 128-token j chunks (16)
NU = J16 // 2               # jc pairs per (head, iblock) unit (8)
IBL = LOCAL // 512          # local 512-token i blocks (2)
ISUB = LOCAL // 128         # local 128-token i sub tiles (8)
LN16 = float(np.log(32.0))

F32 = mybir.dt.float32
BF16 = mybir.dt.bfloat16
FP8 = mybir.dt.float8e4
AF = mybir.ActivationFunctionType
DR = mybir.MatmulPerfMode.DoubleRow


def _slab3(ap2, col0, slab_stride, n):
    """[P, n] view at col0 with an extra middle slab dim [slab_stride, 2]."""
    a = ap2[:, col0:col0 + 1]
    return bass.AP(
        tensor=a.tensor, offset=a.offset,
        ap=[list(a.ap[0]), [slab_stride, 2], [1, n]],
    )


def build_program(affine):
    nc = bacc.Bacc()

    hbt16 = nc.declare_dram_parameter("hbt16", [128, JB * CC * 512], BF16,
                                      isOutput=False)
    hbt8 = nc.declare_dram_parameter("hbt8", [128, JB * CC * 512], FP8,
                                     isOutput=False)
    wq16 = nc.declare_dram_parameter("wq16", [128, CC * D_MODEL], BF16,
                                     isOutput=False)
    wk16 = nc.declare_dram_parameter("wk16", [128, CC * D_MODEL], BF16,
                                     isOutput=False)
    wv8 = nc.declare_dram_parameter("wv8", [128, CC * D_MODEL], FP8,
                                    isOutput=False)
    wo8 = nc.declare_dram_parameter("wo8", [128, CC * D_MODEL], FP8,
                                    isOutput=False)
    wq8p = nc.declare_dram_parameter("wq8p", [128, CC * 128], FP8,
                                     isOutput=False)
    wk8p = nc.declare_dram_parameter("wk8p", [128, CC * 128], FP8,
                                     isOutput=False)
    hb = nc.declare_dram_parameter("hb", [LOCAL, D_MODEL], BF16,
                                   isOutput=False)
    gamma = nc.declare_dram_parameter("gamma", [D_MODEL], F32, isOutput=False)
    beta = nc.declare_dram_parameter("beta", [D_MODEL], F32, isOutput=False)
    out = nc.declare_dram_parameter("out", [LOCAL, D_MODEL], F32,
                                    isOutput=True)

    with tile.TileContext(nc) as tc:
        with (
            tc.tile_pool(name="consts", bufs=1) as consts,
            tc.tile_pool(name="weights", bufs=1) as wpool,
            tc.tile_pool(name="hbt", bufs=1) as hpool,
            tc.tile_pool(name="ktq", bufs=1) as kpool,
            tc.tile_pool(name="vsb", bufs=1) as vpool,
            tc.tile_pool(name="attn", bufs=1) as apool,
            tc.tile_pool(name="exp", bufs=4) as epool,
            tc.tile_pool(name="small", bufs=3) as spool,
            tc.tile_pool(name="xstage", bufs=1) as xpool,
            tc.tile_pool(name="hbres", bufs=2) as rpool,
            tc.tile_pool(name="psum", bufs=2, space="PSUM") as psum,
        ):
            _emit(nc, hbt16, hbt8, wq16, wk16, wv8, wo8, wq8p, wk8p,
                  hb, gamma, beta, out, consts, wpool, hpool, kpool,
                  vpool, apool, epool, spool, xpool, rpool, psum, affine)

    nc.finalize()
    return nc


def _emit(nc, hbt16, hbt8, wq16, wk16, wv8, wo8, wq8p, wk8p, hb, gamma,
          beta, out, consts, wpool, hpool, kpool, vpool, apool, epool,
          spool, xpool, rpool, psum, affine):
    # ---- SBUF tiles ----
    eps_t = consts.tile([128, 1], F32)
    nc.vector.memset(eps_t[:], EPS)
    lnb_t = consts.tile([128, 1], F32)
    nc.vector.memset(lnb_t[:], -LN16)
    if affine:
        gamma_b = consts.tile([128, D_MODEL], F32)
        beta_b = consts.tile([128, D_MODEL], F32)
        g_ap, b_ap = gamma.ap(), beta.ap()
        nc.gpsimd.dma_start(
            out=gamma_b[:],
            in_=bass.AP(tensor=g_ap.tensor, offset=g_ap.offset,
                        ap=[[0, 128], [1, D_MODEL]]))
        nc.gpsimd.dma_start(
            out=beta_b[:],
            in_=bass.AP(tensor=b_ap.tensor, offset=b_ap.offset,
                        ap=[[0, 128], [1, D_MODEL]]))

    wk_sb = wpool.tile([128, CC * D_MODEL], BF16, name="wk")
    wq8_sb = wpool.tile([128, CC * 128], FP8, name="wq8p")
    wk8_sb = wpool.tile([128, CC * 128], FP8, name="wk8p")
    wq_sb = wpool.tile([128, CC * D_MODEL], BF16, name="wq")
    wv_sb = wpool.tile([128, CC * D_MODEL], FP8, name="wv")
    wo_sb = wpool.tile([128, CC * D_MODEL], FP8, name="wo")
    ht16 = hpool.tile([128, JB * CC * 512], BF16, name="ht16")
    ht8 = hpool.tile([128, JB * CC * 512], FP8, name="ht8")
    # kt: per pair [128p = 2 heads x 64 d] x [pair(8) x j(2048)] fp8
    kt = kpool.tile([128, N_PAIR * SEQ], FP8, name="kt")
    qt = kpool.tile([128, N_PAIR * LOCAL], FP8, name="qt")
    # v8: per head 16 j-chunks x (64 dims + ones col + 15 pad)
    # (DoubleRow slab strides must be multiples of 16 elements)
    v8 = vpool.tile([128, N_HEAD * J16 * 80], FP8, name="v8")
    # at: per i-block [128p = pair dims] x [pair(8) x i(512)] fp8
    # (two tiles so the output projection's reads of i-block 0 don't
    # serialize behind i-block-1 normalize writes via tile-granular deps)
    at_ib = [apool.tile([128, 8 * 512], FP8, name=f"at{ib}")
             for ib in range(IBL)]

    # ones columns of v8 (col h*1280 + j*80 + 64)
    for n in range(N_HEAD):
        base = n * (J16 * 80) + 64
        a = v8[:, base:base + 1]
        nc.gpsimd.memset(
            bass.AP(tensor=a.tensor, offset=a.offset,
                    ap=[list(a.ap[0]), [80, J16], [1, 1]]), 1.0)

    # ---- DMA loads ----
    # Single HWDGE (sync) queue, priority order: the DMA engines serialize
    # transfers, so the lead-in critical path (ht16 jb0 -> wk -> wq) goes
    # first; per-chunk weight loads let the projection matmuls accumulate
    # as chunks land.
    def wv_half(half):
        a = wv_sb[:, half * 512:half * 512 + 1]
        dst = bass.AP(tensor=a.tensor, offset=a.offset,
                      ap=[list(a.ap[0]), [1024, CC], [1, 512]])
        s = wv8.ap()
        srcap = bass.AP(tensor=s.tensor, offset=s.offset + half * 512,
                        ap=[list(s.ap[0]), [1024, CC], [1, 512]])
        nc.sync.dma_start(dst, srcap)

    nc.sync.dma_start(ht8[:, ts(0, 4096)], hbt8[:, ts(0, 4096)])
    nc.sync.dma_start(wk8_sb[:], wk8p[:, :])
    nc.sync.dma_start(wq8_sb[:], wq8p[:, :])
    wv_half(0)
    nc.sync.dma_start(ht8[:, ts(1, 4096)], hbt8[:, ts(1, 4096)])
    nc.sync.dma_start(ht8[:, ts(2, 4096)], hbt8[:, ts(2, 4096)])
    nc.sync.dma_start(ht8[:, ts(3, 4096)], hbt8[:, ts(3, 4096)])
    nc.sync.dma_start(ht16[:, ts(0, 4096)], hbt16[:, ts(0, 4096)])
    for c in range(CC):
        nc.sync.dma_start(wk_sb[:, ts(c, 1024)], wk16[:, ts(c, 1024)])
    for c in range(CC):
        nc.sync.dma_start(wq_sb[:, ts(c, 1024)], wq16[:, ts(c, 1024)])
    nc.sync.dma_start(ht16[:, ts(1, 4096)], hbt16[:, ts(1, 4096)])
    nc.sync.dma_start(ht16[:, ts(2, 4096)], hbt16[:, ts(2, 4096)])
    nc.sync.dma_start(ht16[:, ts(3, 4096)], hbt16[:, ts(3, 4096)])
    wv_half(1)
    nc.sync.dma_start(wo_sb[:], wo8[:, :])
    hbres = [rpool.tile([128, D_MODEL], BF16, tag=f"hbres{i % 2}",
                        name=f"hbres{i}") for i in range(ISUB)]
    for i in range(ISUB):
        nc.sync.dma_start(hbres[i][:], hb[ts(i, 128), :])

    # ---- emission helpers ----
    def kq_tile(p, jb, is_q):
        """One [128, 512] projection tile of K^T or Q^T (bf16 matmuls).
        Output partitions = (2 heads of pair p) x 64 dims."""
        w, dst, blk = (wq_sb, qt, LOCAL) if is_q else (wk_sb, kt, SEQ)
        ps = psum.tile([128, 512], F32, tag="proj", name="kqps")
        for c in range(CC):
            nc.tensor.matmul(
                ps[:], w[:, c * 1024 + p * 128: c * 1024 + (p + 1) * 128],
                ht16[:, jb * 4096 + c * 512: jb * 4096 + (c + 1) * 512],
                start=(c == 0), stop=(c == CC - 1),
            )
        nc.vector.tensor_copy(dst[:, p * blk + jb * 512:
                                  p * blk + jb * 512 + 512], ps[:])

    def kq8_tile(jb, is_q):
        """Pair-0 K^T/Q^T from fp8 h and fp8 weights (DoubleRow): tiny
        DMA footprint so attention starts ~13us earlier.  Touches only
        heads 0-1's q/k precision."""
        w, dst, blk = (wq8_sb, qt, LOCAL) if is_q else (wk8_sb, kt, SEQ)
        ps = psum.tile([128, 512], F32, tag="proj", name="kq8ps")
        for cp in range(CC // 2):
            lhs = _slab3(w[:], (2 * cp) * 128, 128, 128)
            rhs = _slab3(ht8[:], jb * 4096 + (2 * cp) * 512, 512, 512)
            nc.tensor.matmul(ps[:], lhs, rhs, start=(cp == 0),
                             stop=(cp == CC // 2 - 1), perf_mode=DR)
        nc.vector.tensor_copy(dst[:, jb * 512: jb * 512 + 512], ps[:])

    def v_tile(j16, half):
        """V projection for one 128-token j-chunk, heads 8h..8h+7 (fp8
        DR)."""
        jb, t0 = divmod(j16, 4)
        ps = psum.tile([128, 512], F32, tag="proj", name="vps")
        for cp in range(CC // 2):
            lhs = _slab3(ht8[:], jb * 4096 + (2 * cp) * 512 + t0 * 128,
                         512, 128)
            rhs = _slab3(wv_sb[:], (2 * cp) * 1024 + half * 512,
                         1024, 512)
            nc.tensor.matmul(ps[:], lhs, rhs, start=(cp == 0),
                             stop=(cp == CC // 2 - 1), perf_mode=DR)
        # strided scatter into v8: head n = 8*half+k gets cols
        # n*1280 + j16*80 .. +64
        base = (8 * half) * (J16 * 80) + j16 * 80
        a = v8[:, base:base + 1]
        dst = bass.AP(tensor=a.tensor, offset=a.offset,
                      ap=[list(a.ap[0]), [J16 * 80, 8], [1, 64]])
        nc.vector.tensor_copy(dst, ps[:])

    acc_of = {}

    def unit(h, ib, u):
        """Scores + exp + PV for head h, i-block ib, jc-pair u."""
        p, hh = divmod(h, 2)
        s4 = psum.tile([128, 1024], F32, tag="s4", name="s4")
        for uu in range(2):
            jc = 2 * u + uu
            lhs = _slab3(kt[ts(hh, 64), :], p * SEQ + jc * 128, 0, 128)
            rhs = _slab3(qt[ts(hh, 64), :], p * LOCAL + ib * 512, 0, 512)
            nc.tensor.matmul(s4[:, ts(uu, 512)], lhs, rhs,
                             start=True, stop=True, perf_mode=DR)
        e = epool.tile([128, 1024], FP8, tag="e", name="e")
        nc.scalar.activation(e[:], s4[:], AF.Exp, bias=lnb_t[:])
        acc = acc_of[(h, ib)]
        lhs = _slab3(v8[:], h * (J16 * 80) + (2 * u) * 80, 80, 65)
        rhs = _slab3(e[:], 0, 512, 512)
        nc.tensor.matmul(acc[0:65, :], lhs, rhs, start=(u == 0),
                         stop=(u == NU - 1), perf_mode=DR)

    def normalize(h, ib):
        acc = acc_of.pop((h, ib))
        rec = spool.tile([1, 512], F32, tag="rec", name="rec")
        nc.vector.reciprocal(rec[:], acc[64:65, :])
        rb = spool.tile([64, 512], F32, tag="recb", name="rb")
        nc.gpsimd.partition_broadcast(rb[:], rec[:])
        p, hh = divmod(h, 2)
        nc.vector.tensor_mul(at_ib[ib][ts(hh, 64), ts(p, 512)],
                             acc[0:64, :], rb[:])

    def pair(p, pre_map=None, post_map=None):
        """Attention for heads 2p, 2p+1 with their units interleaved (the
        two exps per u-slot double the pipeline window for carried work).
        pre_map[(ib, u)] = emitters run BEFORE that u-slot; post_map
        likewise after the slot."""
        pre_map = pre_map or {}
        post_map = post_map or {}
        h0, h1 = 2 * p, 2 * p + 1
        for ib in range(IBL):
            acc_of[(h0, ib)] = psum.tile([128, 512], F32, tag="acc",
                                         name="acc")
            acc_of[(h1, ib)] = psum.tile([128, 512], F32, tag="acc",
                                         name="acc")
            for u in range(NU):
                for fn in pre_map.get((ib, u), ()):
                    fn()
                unit(h0, ib, u)
                unit(h1, ib, u)
                for fn in post_map.get((ib, u), ()):
                    fn()
            normalize(h0, ib)
            normalize(h1, ib)

    # Output blocks in three phases so the per-block Ln/Exp rstd pairs
    # don't thrash the ACT function table against the attention exps:
    # A) projection + residual + bn stats (no ACT), B) one batched
    # Ln/Exp over all 8 variances after the last exp, C) normalize+store.
    x_of = [None] * ISUB
    mv_of = [None] * ISUB
    vbat = consts.tile([128, ISUB], F32, name="vbat")
    rbat = consts.tile([128, ISUB], F32, name="rbat")

    def wo_a(isub):
        ib, t = divmod(isub, 4)
        x = xpool.tile([128, D_MODEL], F32, tag=f"x{isub}", name="x")
        x_of[isub] = x
        for dm in range(2):
            ops = psum.tile([128, 512], F32, tag="proj", name="ops")
            for qp in range(4):
                lhs = _slab3(at_ib[ib][:], (2 * qp) * 512 + t * 128,
                             512, 128)
                rhs = _slab3(wo_sb[:], (2 * qp) * 1024 + dm * 512,
                             1024, 512)
                nc.tensor.matmul(ops[:], lhs, rhs, start=(qp == 0),
                                 stop=(qp == 3), perf_mode=DR)
            nc.vector.tensor_add(x[:, ts(dm, 512)], ops[:],
                                 hbres[isub][:, ts(dm, 512)])
        stats = spool.tile([128, 2, 6], F32, tag="bnst", name="st")
        mv = spool.tile([128, 2], F32, tag=f"bnmv{isub}", name="mv")
        mv_of[isub] = mv
        for gg in range(2):
            nc.vector.bn_stats(stats[:, gg, :], x[:, ts(gg, 512)])
        nc.vector.bn_aggr(mv[:], stats[:])
        nc.vector.tensor_copy(vbat[:, isub:isub + 1], mv[:, 1:2])

    def wo_b():
        nc.scalar.activation(rbat[:], vbat[:], AF.Sqrt, bias=eps_t[:])
        nc.vector.reciprocal(rbat[:], rbat[:])

    def wo_c(isub):
        x = x_of[isub]
        nc.vector.tensor_scalar(
            x[:], x[:], mv_of[isub][:, 0:1], rbat[:, isub:isub + 1],
            op0=mybir.AluOpType.subtract, op1=mybir.AluOpType.mult)
        if affine:
            nc.vector.tensor_mul(x[:], x[:], gamma_b[:])
            nc.vector.tensor_add(x[:], x[:], beta_b[:])
        nc.sync.dma_start(out[ts(isub, 128), :], x[:])

    # ---- schedule ----
    def V(j16, half=0):
        return lambda: v_tile(j16, half)

    def K(p, jb):
        return lambda: kq_tile(p, jb, False)

    def Q(p, ib):
        return lambda: kq_tile(p, ib, True)

    def K8(jb):
        return lambda: kq8_tile(jb, False)

    # Lead-in: minimum work before the first exp can fire (all-fp8 path).
    kq8_tile(0, False)
    kq8_tile(0, True)

    # Each pair self-carries its own later K j-blocks (needed at u-slot
    # 2b) and Q i-block 1, plus the NEXT pair's first K/Q; pair 0 also
    # carries all 16 V chunks (V(2u), V(2u+1) before u-slot u for PV).
    p0_pre = {
        (0, 0): [K8(1), V(0), V(1)],
        (0, 1): [V(2), V(3)],
        (0, 2): [K8(2), V(4), V(5)],
        (0, 3): [V(6), V(7)],
        (0, 4): [K8(3), V(8), V(9)],
        (0, 5): [V(10), V(11)],
        (0, 6): [V(12), V(13)],
        (0, 7): [lambda: kq8_tile(1, True), V(14), V(15)],
        (1, 1): [K(1, 0)],
        (1, 4): [Q(1, 0)],
    }
    pair(0, p0_pre)

    # Pairs 1..7: self-carry K jb 1-3 at u-slots 2,4,6 and Q ib1 at
    # slot 7; hand the next pair its first K/Q during ib 1.  The last
    # pair interleaves the first output-projection blocks into its ib-1
    # phase (their at-deps complete at ib-0's end).
    for p in range(1, N_PAIR):
        pre_map = {
            (0, 2): [K(p, 1)],
            (0, 4): [K(p, 2)],
            (0, 6): [K(p, 3)],
            (0, 7): [Q(p, 1)],
        }
        post_map = {}
        if p in (1, 2):
            # heads 8-15's V chunks, needed from pair 4 on
            for u in range(NU):
                pre_map.setdefault((1, u), []).append(
                    V(8 * (p - 1) + u, 1))
        if p < N_PAIR - 1:
            pre_map.setdefault((1, 1), []).append(K(p + 1, 0))
            pre_map.setdefault((1, 4), []).append(Q(p + 1, 0))
        else:
            for isub in range(4):
                post_map.setdefault((1, isub), []).append(
                    lambda isub=isub: wo_a(isub))
        pair(p, pre_map, post_map)
    for isub in range(4, ISUB):
        wo_a(isub)
    wo_b()
    for isub in range(ISUB):
        wo_c(isub)


_program_cache = {}


def _get_program(affine=False):
    key = ("nc", affine)
    if key not in _program_cache:
        _program_cache[key] = build_program(affine)
    return _program_cache[key]


def _chunk_cols(w):
    """[1024, 1024] -> [128, 8*1024] with col c*1024+m = w[128c+p, m]."""
    return np.ascontiguousarray(
        w.reshape(CC, 128, D_MODEL).transpose(1, 0, 2).reshape(128, -1))


def _h_layout(hp, dt):
    """h_perm [2048, 1024] -> [128, jb(4) x c(8) x 512] in dtype dt."""
    a = hp.astype(dt)
    # [jb, t', c, p] -> [p, jb, c, t']
    a = a.reshape(JB, 512, CC, 128).transpose(3, 0, 2, 1)
    return np.ascontiguousarray(a.reshape(128, -1))


def _shard_inputs(h, Wq, Wkv, Wo, gamma, beta):
    h = np.asarray(h, np.float32)
    Wq = np.asarray(Wq, np.float32)
    Wkv = np.asarray(Wkv, np.float32)
    Wo = np.asarray(Wo, np.float32)
    gamma = np.asarray(gamma, np.float32)
    beta = np.asarray(beta, np.float32)

    # scores DoubleRow contracts the same slab twice -> fold an extra
    # 1/2 into the Wq scale
    scale = 0.5 / np.sqrt(D_HEAD)
    Wk = Wkv[:, :N_HEAD * D_HEAD]
    Wv = Wkv[:, N_HEAD * D_HEAD:]
    wq16 = _chunk_cols(Wq * scale).astype(ml_dtypes.bfloat16)
    wk16 = _chunk_cols(Wk).astype(ml_dtypes.bfloat16)
    wq8p = np.ascontiguousarray(
        (Wq[:, :128] * scale).reshape(CC, 128, 128).transpose(1, 0, 2)
        .reshape(128, -1)).astype(ml_dtypes.float8_e4m3)
    wk8p = np.ascontiguousarray(
        Wk[:, :128].reshape(CC, 128, 128).transpose(1, 0, 2)
        .reshape(128, -1)).astype(ml_dtypes.float8_e4m3)
    wv8 = _chunk_cols(Wv).astype(ml_dtypes.float8_e4m3)
    wo8 = _chunk_cols(Wo).astype(ml_dtypes.float8_e4m3)

    in_maps = []
    for core in range(N_CORES):
        b, r = divmod(core, 2)
        hb_full = h[:, b, :]
        if r == 0:
            hp = hb_full
        else:
            hp = np.concatenate([hb_full[LOCAL:], hb_full[:LOCAL]], axis=0)
        in_maps.append({
            "hbt16": _h_layout(hp, ml_dtypes.bfloat16),
            "hbt8": _h_layout(hp, ml_dtypes.float8_e4m3),
            "wq16": wq16, "wk16": wk16, "wv8": wv8, "wo8": wo8,
            "wq8p": wq8p, "wk8p": wk8p,
            "hb": np.ascontiguousarray(hp[:LOCAL].astype(ml_dtypes.bfloat16)),
            "gamma": gamma, "beta": beta,
        })
    return in_maps


def kernel(h, Wq, Wkv, Wo, gamma, beta, _trace=False):
    gamma = np.asarray(gamma, np.float32)
    beta = np.asarray(beta, np.float32)
    affine = not (np.all(gamma == 1.0) and np.all(beta == 0.0))
    nc = _get_program(affine)
    in_maps = _shard_inputs(h, Wq, Wkv, Wo, gamma, beta)
    res = run_bass_kernel_spmd(nc, in_maps, list(range(N_CORES)),
                               trace=_trace)
    if _trace:
        kernel.last_results = res

    out = np.empty((SEQ, BATCH, D_MODEL), np.float32)
    for core in range(N_CORES):
        b, r = divmod(core, 2)
        out[r * LOCAL:(r + 1) * LOCAL, b, :] = res.results[core]["out"]
    return out
